# revision 52
# baseline (speedup 1.0000x reference)
"""Trainium2 Bass kernel for nn_BoundarySuppressionWithSmoothing.

Full inputs: x [8,1,512,1024] f32, prediction [8,1,512,1024] int32.
Sharding: pure data parallel, image i -> core i.

Per-core algorithm (image I [512,1024], layout A: 4 row-chunks of [128,1024]):
  - boundary detection via exp-encoded morphology on PE + ACT (exp/ln-free
    product compare), masks m3..m0 via a mask-carried dilation chain
  - 4 iterations of masked 3x3 box average with replication padding
  - separable dilated 7x7 Gaussian (dilation 6) via PE banded matmuls

Engine balance (v2): DVE was the bottleneck, so the count reciprocal runs
as an ACT spline recip, the old DVE select-mask add is folded into the
count matmul (pn = box9(m) - 16*m, Mk = relu(-2*pn+1) as int16 on ACT),
the mask H-presums run on the Pool/GPSIMD engine, and the mask/count
pipeline for iteration it+1 is emitted one iteration ahead so PE/ACT/Pool
work it while DVE finishes iteration it's value ops. The horizontal
gaussian for each chunk is emitted as soon as that chunk's final select
lands, overlapping the U-loop tail.

Host I/O is compressed for the axon tunnel: x ships as fp16, prediction as
int8, y returns as fp16 (converted back to f32 host-side). The value path
runs in fp16 on-device (DVE 2-byte fast modes); the mask/count path stays
bf16 (exact small ints). The compiled executable, weight pack, and output
scratch buffer are cached device-resident so warm calls only move x/pred
in and y out.
"""
import math
import sys
from contextlib import ExitStack

import numpy as np

sys.path.insert(0, '/opt/trn_rl_repo')

import concourse.bass as bass  # noqa: E402
import concourse.bacc as bacc  # noqa: E402
import concourse.tile as tile  # noqa: E402
from concourse import mybir  # noqa: E402

P = 128
W = 1024
H = 512
CH = 4          # row chunks
B = 8           # batch == cores
ALPHA = 4.6     # morphology exp-encoding scale
PTHR = float(np.exp(4.2))   # product threshold for boundary test
DT = mybir.dt
AF = mybir.ActivationFunctionType
OP = mybir.AluOpType


# ---------------------------------------------------------------- weights ---
def _gauss1d():
    size, sigma = 7, 1.0
    u = np.exp(-((np.arange(size) - 3.0) ** 2) / (2 * sigma ** 2))
    # 2D reference kernel is outer(u,u)/sum => separable 1D = u/sum(u)
    return (u / u.sum()).astype(np.float64)


def build_host_consts():
    """All constant weight matrices, as one dict of fp32 arrays [128,x]."""
    c = {}
    tri = np.zeros((P, P), np.float32)
    for k in range(P):
        for d in (-1, 0, 1):
            if 0 <= k + d < P:
                tri[k, k + d] = 1.0   # lhsT[k,m]: out m from in k, |k-m|<=1
    c['T_mid'] = tri
    t_top = tri.copy(); t_top[0, 0] = 2.0
    c['T_top'] = t_top
    t_bot = tri.copy(); t_bot[P - 1, P - 1] = 2.0
    c['T_bot'] = t_bot
    t_up = np.zeros((P, P), np.float32); t_up[P - 1, 0] = 1.0
    c['T_up'] = t_up
    t_dn = np.zeros((P, P), np.float32); t_dn[0, P - 1] = 1.0
    c['T_dn'] = t_dn
    c['I'] = np.eye(P, dtype=np.float32)
    c['M16'] = (-16.0 * np.eye(P)).astype(np.float32)
    bvec = np.zeros((P, P), np.float32)
    bvec[:, 0] = -4.0; bvec[0, 0] = -3.0      # bv_top
    bvec[:, 1] = -4.0; bvec[P - 1, 1] = -3.0  # bv_bot
    c['BVEC'] = bvec

    g = _gauss1d()
    for j in range(7):
        c[f'G{j}'] = (np.eye(P) * g[j]).astype(np.float16).astype(np.float32)
    # vertical gaussian: Wv[R,S] = sum_j g[j] [clamp(R+6(j-3),0,H-1)==S]
    Wv = np.zeros((H, H), np.float64)
    for R in range(H):
        for j in range(7):
            S = min(max(R + 6 * (j - 3), 0), H - 1)
            Wv[R, S] += g[j]
    for c_dst in range(CH):
        for c_src in range(CH):
            if abs(c_dst - c_src) > 1:
                continue
            blk = Wv[c_dst * P:(c_dst + 1) * P, c_src * P:(c_src + 1) * P]
            if not blk.any():
                continue
            # lhsT[k,m] = Wv[dst=128c+m, src=128c'+k]
            c[f'B_{c_dst}_{c_src}'] = (
                np.ascontiguousarray(blk.T).astype(np.float16).astype(np.float32))
    return c


# phase-M-critical weights packed first so a split wstage DMA lands them
# early; must match between _emit_once and _build_program
CRIT_W = ('I', 'T_mid', 'T_up', 'T_dn', 'T_top', 'T_bot', 'BVEC')


def _worder(consts):
    rest = sorted(n for n in consts.keys() if n not in CRIT_W)
    return list(CRIT_W) + rest


# ----------------------------------------------------------------- kernel ---
def build_kernel(ctx: ExitStack, tc: "tile.TileContext", outs, ins, reps=1):
    for _ in range(reps):
        _emit_once(ctx, tc, outs, ins)


def _emit_once(ctx: ExitStack, tc: "tile.TileContext", outs, ins):
    nc = tc.nc
    y = outs[0]                       # [512,1024] fp16 DRAM
    x, pred, wpack = ins              # x fp16, pred int8, wpack fp16 DRAM

    consts = build_host_consts()
    wnames = _worder(consts)

    if not hasattr(tc, '_bs_pools'):
        tc._bs_pools = (
            ctx.enter_context(tc.tile_pool(name="sb", bufs=1)),
            ctx.enter_context(tc.tile_pool(name="sbR", bufs=3)),
            ctx.enter_context(tc.tile_pool(name="sbM", bufs=2)),
            ctx.enter_context(tc.tile_pool(name="wp", bufs=1)),
            ctx.enter_context(tc.tile_pool(name="psB", bufs=2, space="PSUM")),
            ctx.enter_context(tc.tile_pool(name="psY", bufs=2, space="PSUM")))
    sb, sbR, sbM, wpool, psB, psY = tc._bs_pools

    # ---- persistent image buffers (chunk-blocked big tiles: one DMA each) ----
    OAbig = sb.tile([P, CH * W], DT.float16, name="OAbig", tag="OAbig")
    OBbig = sb.tile([P, CH * W], DT.float16, name="OBbig", tag="OBbig")
    OA = [OAbig[:, c * W:(c + 1) * W] for c in range(CH)]
    OB = [OBbig[:, c * W:(c + 1) * W] for c in range(CH)]
    # DMA order = consumption order: prediction feeds the phase-M exps
    # immediately (split so chunk 0's exp starts after the first half
    # lands); weights next; x is only needed at the first U iteration.
    pvbig = OBbig[:].bitcast(DT.int8)[:, 0:CH * W]
    HW2 = CH * W // 2
    nc.sync.dma_start(pvbig[:, 0:HW2], pred[:, 0:HW2])
    nc.sync.dma_start(pvbig[:, HW2:], pred[:, HW2:])
    # ACT reads the int8 labels directly in the Exp encode (no f32 staging);
    # the lab{c} tags still back the gaussian gs buffers later
    lab = [pvbig[:, c * W:(c + 1) * W] for c in range(CH)]

    # ---- load + prepare weights ----
    # split DMA: the critical block (packed first, see _worder) lands in
    # ~0.6us so PE isn't gated on the full 2.2us weight transfer
    wstage = sb.tile([P, len(wnames) * P], DT.float16, tag="wstage")
    NCRIT = len(CRIT_W) * P
    nc.sync.dma_start(wstage[:, 0:NCRIT], wpack[:, 0:NCRIT])
    nc.sync.dma_start(wstage[:, NCRIT:], wpack[:, NCRIT:len(wnames) * P])
    nc.sync.dma_start(OAbig[:], x[:, :])
    wt = {}
    BF16_W = {'T_mid', 'T_top', 'T_bot', 'T_up', 'T_dn', 'I', 'M16'}
    # phase-M-critical weights first, on DVE (idle at startup; ~94ns each)
    # so PE isn't gated on Pool's serial Q7 copy stream; everything needed
    # later (M16, R_*, G*, B_*) goes to Pool in first-use order
    for name in CRIT_W:
        if name == 'BVEC':
            continue
        i = wnames.index(name)
        t = wpool.tile([P, P], DT.bfloat16, name=f"w_{name}", tag=f"w_{name}")
        nc.vector.tensor_copy(t[:], wstage[:, i * P:(i + 1) * P])
        wt[name] = t
    for name in wnames:
        if name in CRIT_W:
            continue
        i = wnames.index(name)
        dt_w = DT.bfloat16 if name in BF16_W else DT.float16
        t = wpool.tile([P, P], dt_w, name=f"w_{name}", tag=f"w_{name}")
        nc.gpsimd.tensor_copy(t[:], wstage[:, i * P:(i + 1) * P])
        wt[name] = t
    # fp16 variants of vertical matrices for the value path
    for name in ('T_mid', 'T_top', 'T_bot', 'T_up', 'T_dn'):
        t = wpool.tile([P, P], DT.float16, name=f"wr_{name}", tag=f"wr_{name}")
        i = wnames.index(name)
        nc.gpsimd.tensor_copy(t[:], wstage[:, i * P:(i + 1) * P])
        wt['R' + name[1:]] = t

    def TRv(c):
        return wt['T_top'] if c == 0 else (wt['T_bot'] if c == CH - 1 else wt['T_mid'])

    def Rv(c):
        return wt['R_top'] if c == 0 else (wt['R_bot'] if c == CH - 1 else wt['R_mid'])

    # ---- const bias vectors ----
    def make_const(val, tag):
        t = sb.tile([P, 1], DT.float32, tag=tag)
        nc.vector.memset(t[:], val)
        return t

    b_enc_max = make_const(-9.0 * ALPHA, "b_enc_max")
    b_enc_min = make_const(+9.0 * ALPHA, "b_enc_min")
    bv_mid = make_const(-4.0, "bv_mid")
    ib = wnames.index('BVEC')
    bv_top = sb.tile([P, 1], DT.float32, name="bv_top", tag="bv_top")
    nc.vector.tensor_copy(bv_top[:], wstage[:, ib * P:ib * P + 1])
    bv_bot = sb.tile([P, 1], DT.float32, name="bv_bot", tag="bv_bot")
    nc.vector.tensor_copy(bv_bot[:], wstage[:, ib * P + 1:ib * P + 2])
    one_c = make_const(1.0, "one_c")

    def bv(c):
        return bv_top if c == 0 else (bv_bot if c == CH - 1 else bv_mid)

    GW = W + 2

    def c3(ap, cw=GW):
        # [P, CH*cw] 2D AP -> [P, CH, cw] chunk-major 3D view
        return ap.rearrange("p (c w) -> p c w", c=CH)

    def gtile(tag, dtype, guard_val, pool=sb):
        # one [P, CH*GW] tile per family: chunk views + strided guard memsets
        big = pool.tile([P, CH * GW], dtype, name=tag, tag=tag)
        nc.gpsimd.memset(big[:, 0:CH * GW:GW], guard_val)
        nc.gpsimd.memset(big[:, GW - 1:CH * GW:GW], guard_val)
        return big, [big[:, c * GW:(c + 1) * GW] for c in range(CH)]

    EmaxB, Emax = gtile("Emax", DT.bfloat16, 0.0)
    EminB, Emin = gtile("Emin", DT.bfloat16, 0.0)
    mPairs = [gtile(f"m{i}_", DT.bfloat16, 1.0) for i in range(4)]
    mB = [p[0] for p in mPairs]
    m = [p[1] for p in mPairs]
    xmB, xm = gtile("xm", DT.float16, 0.0)
    HNB = sb.tile([P, CH * W], DT.bfloat16, name="HNB", tag="HMaB")
    HN = [HNB[:, c * W:(c + 1) * W] for c in range(CH)]
    HMaB = sb.tile([P, CH * W], DT.bfloat16, name="HMaB", tag="HMaB")
    HMa = [HMaB[:, c * W:(c + 1) * W] for c in range(CH)]
    hlrB = sb.tile([P, CH * W], DT.float16, name="hlrB", tag="hlrB")
    hlr = [hlrB[:, c * W:(c + 1) * W] for c in range(CH)]

    def data(t):
        return t[:, 1:W + 1]

    def shl(t):
        return t[:, 0:W]

    def shr(t):
        return t[:, 2:W + 2]

    def pool_copy_predicated(out, mask, dat):
        eng = nc.gpsimd
        eng.add_instruction(mybir.InstCopyPredicated(
            name=f"I-{eng.bass.next_id()}",
            ins=[eng.lower_ap(mask), eng.lower_ap(dat)],
            outs=[eng.lower_ap(out)]))

    def act_recip(out, in_, bias):
        # ACT spline reciprocal: plenty accurate for 1/n of exact small
        # counts (the bass wrapper refuses Reciprocal outright, so emit
        # the instruction directly)
        eng = nc.scalar
        imm = lambda v: mybir.ImmediateValue(dtype=DT.float32, value=v)
        eng.add_instruction(mybir.InstActivation(
            name=eng.bass.get_next_instruction_name(),
            func=AF.Reciprocal,
            ins=[eng.lower_ap(in_), imm(bias), imm(1.0), imm(0.0)],
            outs=[eng.lower_ap(out)]))

    def mm_group(pt, pairs):
        # split into N=512 sub-matmuls (PSUM bank limit); weight-major order
        # so consecutive matmuls share the stationary operand (fewer LDW).
        n = pt.shape[1]
        halves = list(range(0, n, 512))
        for i, (lhsT, rhs) in enumerate(pairs):
            for h0 in halves:
                nc.tensor.matmul(pt[:, h0:h0 + 512], lhsT,
                                 rhs[:, h0:h0 + 512], start=(i == 0),
                                 stop=(i == len(pairs) - 1))

    # ================= Phase M: encode + boundary masks ===================
    for c in range(CH):
        nc.scalar.activation(data(Emax[c]), lab[c], AF.Exp,
                             bias=b_enc_max[:], scale=ALPHA)
        nc.scalar.activation(data(Emin[c]), lab[c], AF.Exp,
                             bias=b_enc_min[:], scale=-ALPHA)
    # horizontal presums (DVE, bf16 fast mode)
    SX = [sb.tile([P, W], DT.bfloat16, name=f"SX{c}", tag=f"SX{c}") for c in range(CH)]
    for c in range(CH):
        nc.vector.tensor_tensor(HN[c][:], shl(Emin[c]), shr(Emin[c]), op=OP.add)
        nc.vector.tensor_tensor(HN[c][:], HN[c][:], data(Emin[c]), op=OP.add)
        nc.vector.tensor_tensor(SX[c][:], shl(Emax[c]), shr(Emax[c]), op=OP.add)
    for c in range(CH):
        p1 = psB.tile([P, W], DT.float32, name="pS1", tag="psb")
        pairs = [(wt['T_mid'][:], data(Emax[c])),
                 (wt['I'][:], SX[c][:])]
        if c > 0:
            pairs.append((wt['T_up'][:], data(Emax[c - 1])))
        if c < CH - 1:
            pairs.append((wt['T_dn'][:], data(Emax[c + 1])))
        mm_group(p1[:], pairs)
        sc1 = sbR.tile([P, W], DT.bfloat16, name="sc1", tag="nb")
        nc.scalar.copy(sc1[:], p1[:])

        p2 = psB.tile([P, W], DT.float32, name="pS2", tag="psb")
        pairs = [(wt['T_mid'][:], HN[c][:])]
        if c > 0:
            pairs.append((wt['T_up'][:], HN[c - 1][:]))
        if c < CH - 1:
            pairs.append((wt['T_dn'][:], HN[c + 1][:]))
        mm_group(p2[:], pairs)
        pb = sbR.tile([P, W], DT.bfloat16, name="pb", tag="zt")
        nc.vector.tensor_tensor(pb[:], sc1[:], p2[:], op=OP.mult)
        nc.vector.tensor_scalar(data(m[3][c]), pb[:], PTHR, None, op0=OP.is_lt)

    # ================= Chain: m3 -> m2 -> m1 -> m0 ========================
    # (erosion semantics need guard cols = 1.0 while a mask is a chain input;
    # after its last chain use, guards are replicated for the U loop's
    # replication-padded box sums)
    for k in range(3):
        mp, mn = m[3 - k], m[2 - k]
        for c in range(CH):
            sm = sbR.tile([P, W], DT.bfloat16, name="sm", tag="sm")
            nc.vector.tensor_tensor(sm[:], shl(mp[c]), shr(mp[c]), op=OP.add)
            ps = psB.tile([P, W], DT.float32, name="pCh", tag="psb")
            pairs = [(wt['T_mid'][:], data(mp[c])),
                     (wt['I'][:], sm[:])]
            if c > 0:
                pairs.append((wt['T_up'][:], data(mp[c - 1])))
            if c < CH - 1:
                pairs.append((wt['T_dn'][:], data(mp[c + 1])))
            mm_group(ps[:], pairs)
            nc.scalar.activation(data(mn[c]), ps[:], AF.Relu, bias=bv(c)[:],
                                 scale=1.0)
        # mp fully consumed: replicate guards for the U loop (one strided
        # copy per side covers all four chunks)
        mpB = mB[3 - k]
        nc.vector.tensor_copy(mpB[:, 0:CH * GW:GW], mpB[:, 1:CH * GW:GW])
        nc.vector.tensor_copy(mpB[:, GW - 1:CH * GW:GW], mpB[:, W:CH * GW:GW])
    nc.vector.tensor_copy(mB[0][:, 0:CH * GW:GW], mB[0][:, 1:CH * GW:GW])
    nc.vector.tensor_copy(mB[0][:, GW - 1:CH * GW:GW], mB[0][:, W:CH * GW:GW])

    # ================= U loop =============================================
    GA = 18
    gs = [sb.tile([P, W + 2 * GA], DT.float16, name=f"gs{c}", tag=f"lab{c}")
          for c in range(CH)]
    hg = [sb.tile([P, W], DT.float16, name=f"Emin{c}", tag=f"Emin{c}") for c in range(CH)]
    yo = OB  # OBbig is free after the last U iteration; one output DMA

    def emit_gauss_h(c, src):
        # horizontal dilated gaussian for chunk c, emitted as soon as the
        # final U-iteration output for c lands (overlaps the U-loop tail)
        nc.vector.tensor_copy(gs[c][:, GA:GA + W], src)
        nc.vector.tensor_copy(gs[c][:, 0:GA], src[:, 0:1].to_broadcast((P, GA)))
        nc.vector.tensor_copy(gs[c][:, GA + W:],
                              src[:, W - 1:W].to_broadcast((P, GA)))
        # psb pool: the U loop's count tiles are retired by the time the
        # tail gaussian runs, so this doesn't collide with the value
        # matmuls' psy rotation
        ph = psB.tile([P, W], DT.float32, name="pH", tag="psb")
        for h in range(2):
            for j in range(7):
                off = GA + 6 * (j - 3) + h * 512
                nc.tensor.matmul(ph[:, h * 512:(h + 1) * 512], wt[f'G{j}'][:],
                                 gs[c][:, off:off + 512],
                                 start=(j == 0), stop=(j == 6))
        nc.scalar.copy(hg[c][:], ph[:])

    def emit_gauss_v(c):
        pv = psY.tile([P, W], DT.float32, name="pV", tag="psy")
        srcs = [cc for cc in range(CH) if f'B_{c}_{cc}' in wt]
        mm_group(pv[:], [(wt[f'B_{c}_{cc}'][:], hg[cc][:]) for cc in srcs])
        nc.scalar.copy(yo[c], pv[:])
        nc.sync.dma_start(y[:, c * W:(c + 1) * W], OBbig[:, c * W:(c + 1) * W])

    # chunk-merged 3D views for the U loop's elementwise stages
    xm3 = c3(xmB[:])
    xm_c = xm3[:, :, 1:W + 1]
    xm_l, xm_r = xm3[:, :, 0:W], xm3[:, :, 2:W + 2]
    hlr3 = c3(hlrB[:], W)
    HMa3 = c3(HMaB[:], W)
    m3v = [c3(t[:]) for t in mB]

    def emit_mask_path(it):
        # counts/select-mask pipeline for iteration `it`: depends ONLY on
        # the mask m[it], so it is emitted one iteration AHEAD of the value
        # path -- PE/ACT/Pool chew on it while DVE finishes the previous
        # iteration's value ops.
        mi = m[it]
        for c in range(CH):
            # Pool does shl+shr (stock Q7 op); DVE adds the center in place
            # (halved: smaller per-op DVE drains)
            nc.gpsimd.tensor_tensor(HMa[c][:], shl(mi[c]), shr(mi[c]), op=OP.add)
            for h0 in (0, 512):
                sl = slice(h0, h0 + 512)
                sg = slice(h0 + 1, h0 + 513)
                nc.vector.tensor_tensor(HMa[c][:, sl], HMa[c][:, sl],
                                        mi[c][:, sg], op=OP.add)
        MkL, nbL = [], []
        for c in range(CH):
            pn = psB.tile([P, W], DT.float32, name="pN", tag="psb")
            pairs = [(TRv(c)[:], HMa[c][:]),
                     (wt['M16'][:], data(mi[c]))]  # pn = box9(m) - 16*m
            if c > 0:
                pairs.append((wt['T_up'][:], HMa[c - 1][:]))
            if c < CH - 1:
                pairs.append((wt['T_dn'][:], HMa[c + 1][:]))
            mm_group(pn[:], pairs)
            # Mk = relu(-2*pn + 1): nonzero exactly where m==1 (pn<=-7) or
            # n==0 (pn==0); zero where m==0, n>=1 (pn>=1). Exact small ints,
            # so the int16 output (copy_predicated wants an integer mask)
            # is lossless. One ACT op replaces the old zt + DVE mask-add.
            Mk = sbM.tile([P, W], DT.int16, name="Mk", tag=f"Mk{c}")
            nc.scalar.activation(Mk[:], pn[:], AF.Relu, bias=1.0, scale=-2.0)
            MkL.append(Mk)
            # nb = 1/(pn + eps) on ACT: correct 1/n where m==0 and n>=1;
            # garbage-but-finite elsewhere (those pixels are overwritten by
            # the predicated copy below). eps keeps n==0 in the valid range.
            nb = sbM.tile([P, W], DT.float16, name="nb", tag=f"nb{c}")
            act_recip(nb[:], pn[:], 2.0 ** -40)
            nbL.append(nb)
        return MkL, nbL

    def emit_val_prep(c, src, mi):
        # xm = src*m and hlr = H3(xm) for one chunk, in 512-col halves:
        # the HW DVE pays a pipeline DRAIN ~ (dur-266ns) per op, so two
        # small drains beat one big one, and each PE half-matmul can start
        # a half earlier
        for h0 in (0, 512):
            sl = slice(h0, h0 + 512)            # W-indexed (src, hlr)
            sg = slice(h0 + 1, h0 + 513)        # xm/m data cols
            nc.vector.tensor_tensor(xm[c][:, sg], src[c][:, sl],
                                    mi[c][:, sg], op=OP.mult)
        for h0 in (0, 512):
            sl = slice(h0, h0 + 512)
            nc.vector.tensor_tensor(hlr[c][:, sl], xm[c][:, h0:h0 + 512],
                                    xm[c][:, h0 + 2:h0 + 514], op=OP.add)
        nc.vector.tensor_tensor(hlr[c][:, 0:1], hlr[c][:, 0:1],
                                xm[c][:, 1:2], op=OP.add)
        nc.vector.tensor_tensor(hlr[c][:, W - 1:W], hlr[c][:, W - 1:W],
                                xm[c][:, W:W + 1], op=OP.add)
        for h0 in (0, 512):
            sl = slice(h0, h0 + 512)
            sg = slice(h0 + 1, h0 + 513)
            nc.vector.tensor_tensor(hlr[c][:, sl], hlr[c][:, sl],
                                    xm[c][:, sg], op=OP.add)

    cur, nxt = OA, OB
    mk_nb = emit_mask_path(0)
    for c in range(CH):
        emit_val_prep(c, cur, m[0])
    for it in range(4):
        mi = m[it]
        MkL, nbL = mk_nb
        ysbL = []
        for c in range(CH):
            pyt = psY.tile([P, W], DT.float32, name="pY", tag="psy")
            pairs = [(Rv(c)[:], hlr[c][:])]
            if c > 0:
                pairs.append((wt['R_up'][:], hlr[c - 1][:]))
            if c < CH - 1:
                pairs.append((wt['R_dn'][:], hlr[c + 1][:]))
            mm_group(pyt[:], pairs)
            # avg = Y * (1/n); n==0 -> garbage, overwritten below. ACT
            # drains Y to SBUF, DVE multiplies (2-byte SBUF fast mode).
            ysb = sbR.tile([P, W], DT.float16, name="ysb", tag=f"ysb{c % 2}")
            nc.scalar.copy(ysb[:], pyt[:])
            ysbL.append(ysb)
        if it < 3:
            # next iteration's mask path, emitted here so it lands in the
            # engine queues behind this iteration's matmuls/drains
            mk_nb = emit_mask_path(it + 1)
        for c in range(CH):
            for h0 in (0, 512):
                sl = slice(h0, h0 + 512)
                nc.vector.tensor_tensor(nxt[c][:, sl], ysbL[c][:, sl],
                                        nbL[c][:, sl], op=OP.mult)
                nc.vector.copy_predicated(nxt[c][:, sl], MkL[c][:, sl],
                                          cur[c][:, sl])
            if it < 3:
                # software pipeline: this chunk's next-iteration xm/hlr
                # right after its select, so PE's next value group unblocks
                # after ~2 chunks instead of all four
                emit_val_prep(c, nxt, m[it + 1])
            else:
                # overlap the horizontal gaussian with the U-loop tail:
                # chunk c's result is final as soon as its select lands;
                # each vertical group follows as soon as its band of hg
                # rows exists, and the output DMA streams out per chunk
                emit_gauss_h(c, nxt[c])
                if c >= 1:
                    emit_gauss_v(c - 1)
        cur, nxt = nxt, cur
    emit_gauss_v(CH - 1)


# ------------------------------------------------------------ host driver ---
_CACHE = {}


def _build_program(reps=1):
    key = ('nc', reps)
    if key in _CACHE:
        return _CACHE[key], _CACHE['wpack']
    consts = build_host_consts()
    wnames = _worder(consts)
    # fp16 pack: every weight is 16-bit on device anyway; values are exact
    # small ints or already fp16-rounded, so no precision loss
    wpack = np.zeros((P, len(wnames) * P), np.float16)
    for i, n in enumerate(wnames):
        wpack[:, i * P:(i + 1) * P] = consts[n].astype(np.float16)

    nc = bacc.Bacc("TRN2", target_bir_lowering=False, debug=False,
                   num_devices=B)
    x_d = nc.dram_tensor("x", [P, CH * W], DT.float16,
                         kind="ExternalInput").ap()
    p_d = nc.dram_tensor("prediction", [P, CH * W], DT.int8,
                         kind="ExternalInput").ap()
    w_d = nc.dram_tensor("wpack", list(wpack.shape), DT.float16,
                         kind="ExternalInput").ap()
    y_d = nc.dram_tensor("y", [P, CH * W], DT.float16,
                         kind="ExternalOutput").ap()
    with tile.TileContext(nc) as tc:
        with ExitStack() as ctx:
            build_kernel(ctx, tc, [y_d], [x_d, p_d, w_d], reps=reps)
    nc.compile()
    _CACHE[('nc', reps)] = nc
    _CACHE['wpack'] = wpack
    return nc, wpack


def _get_exec(reps=1):
    """Compile (once) the 8-core sharded executable; stage constants."""
    key = ('exec', reps)
    if key in _CACHE:
        return _CACHE[key]
    import jax
    from jax.sharding import Mesh, PartitionSpec, NamedSharding
    from jax.experimental.shard_map import shard_map
    from concourse import bass2jax

    bass2jax.install_neuronx_cc_hook()
    nc, wpack = _build_program(reps)

    partition_name = (nc.partition_id_tensor.name
                      if nc.partition_id_tensor else None)
    in_names, out_names, out_avals = [], [], []
    for alloc in nc.m.functions[0].allocations:
        if not isinstance(alloc, mybir.MemoryLocationSet):
            continue
        name = alloc.memorylocations[0].name
        if alloc.kind == "ExternalInput":
            if name != partition_name:
                in_names.append(name)
        elif alloc.kind == "ExternalOutput":
            out_names.append(name)
            out_avals.append(jax.core.ShapedArray(
                tuple(alloc.tensor_shape), mybir.dt.np(alloc.dtype)))
    n_params = len(in_names)
    n_outs = len(out_names)

    devices = jax.devices()[:B]
    mesh = Mesh(np.asarray(devices), ("core",))
    shard = NamedSharding(mesh, PartitionSpec("core"))
    assert in_names == ['x', 'prediction', 'wpack'], in_names
    base_shapes = [
        jax.ShapeDtypeStruct((B * P, CH * W), np.float16, sharding=shard),
        jax.ShapeDtypeStruct((B * P, CH * W), np.int8, sharding=shard),
        jax.ShapeDtypeStruct((B * wpack.shape[0], wpack.shape[1]), np.float16,
                             sharding=shard),
    ]
    y_shape = jax.ShapeDtypeStruct((B * P, CH * W), np.float16, sharding=shard)

    # Content-address the jitted function name: the axon-side executable
    # cache can serve a stale NEFF for an unchanged module name ("jit__body")
    # even when the embedded BIR changed, so bake the program hash into the
    # module name to force an honest compile per kernel version.
    import hashlib
    bir_tag = hashlib.sha1(nc.to_json_bytes()).hexdigest()[:10]

    def make_compile_fn(with_y):
        # the kernel writes every y element, so the zero-filled y input
        # operand (run_bass_via_pjrt's donation scheme) is droppable if the
        # lowering accepts an output with no matching input operand
        all_names = list(in_names) + (list(out_names) if with_y else [])
        if partition_name is not None:
            all_names.append(partition_name)

        def _body(*args):
            operands = list(args)
            if partition_name is not None:
                operands.append(bass2jax.partition_id_tensor())
            outs = bass2jax._bass_exec_p.bind(
                *operands, out_avals=tuple(out_avals),
                in_names=tuple(all_names), out_names=tuple(out_names),
                lowering_input_output_aliases=(),
                sim_require_finite=True, sim_require_nnan=True, nc=nc)
            return tuple(outs)

        _body.__name__ = f"_body_{bir_tag}"
        _body.__qualname__ = _body.__name__
        nin = n_params + (n_outs if with_y else 0)
        arg_shapes = base_shapes + ([y_shape] * n_outs if with_y else [])

        def compile_fn():
            jf = jax.jit(shard_map(
                _body, mesh=mesh,
                in_specs=(PartitionSpec("core"),) * nin,
                out_specs=(PartitionSpec("core"),) * n_outs,
                check_rep=False), keep_unused=True)
            return jf.lower(*arg_shapes).compile()
        return compile_fn

    with_y = False
    try:
        compiled = bass2jax.fast_dispatch_compile(make_compile_fn(False))
    except Exception:
        with_y = True
        try:
            compiled = bass2jax.fast_dispatch_compile(make_compile_fn(True))
        except Exception:
            compiled = make_compile_fn(True)()

    wd = jax.device_put(np.concatenate([wpack] * B, axis=0), shard)
    extra = (wd,)
    zd = None
    if with_y:
        zd = jax.device_put(np.zeros((B * H, W), np.float16), shard)
        extra = (wd, zd)
    jax.block_until_ready(extra)

    st = {'compiled': compiled, 'shard': shard, 'wd': wd, 'zd': zd,
          'extra': extra, 'with_y': with_y, 'nc': nc, 'wpack': wpack}
    _CACHE[('exec', reps)] = st
    return st


def _stage_inputs(x, prediction):
    """Host-compress + device_put with the executable's sharding."""
    import jax
    st = _get_exec()
    # chunk-blocked per-core layout [P, CH*W]: row p holds chunks side by side
    xs = np.ascontiguousarray(
        x.reshape(B, CH, P, W).transpose(0, 2, 1, 3).reshape(B * P, CH * W)
    ).astype(np.float16)
    ps = np.ascontiguousarray(
        prediction.reshape(B, CH, P, W).transpose(0, 2, 1, 3)
        .reshape(B * P, CH * W)).astype(np.int8)
    xd = jax.device_put(xs, st['shard'])
    pd = jax.device_put(ps, st['shard'])
    return xd, pd


def _unpack_y(arr):
    """[B*P, CH*W] fp16 chunk-blocked -> [B,1,H,W] f32."""
    return (np.asarray(arr).astype(np.float32)
            .reshape(B, P, CH, W).transpose(0, 2, 1, 3).reshape(B, 1, H, W))


def kernel(x: np.ndarray, prediction: np.ndarray) -> np.ndarray:
    st = _get_exec()
    xd, pd = _stage_inputs(x, prediction)
    out = st['compiled'](xd, pd, *st['extra'])
    return _unpack_y(out[0])


if __name__ == "__main__":
    xs = np.random.randn(B, 1, H, W).astype(np.float32)
    ps = np.random.randint(0, 19, size=(B, 1, H, W)).astype(np.int32)
    print(kernel(xs, ps).shape)



# revision 54
# speedup vs baseline: 1.0660x; 1.0660x over previous
"""Trainium2 Bass kernel for nn_BoundarySuppressionWithSmoothing.

Full inputs: x [8,1,512,1024] f32, prediction [8,1,512,1024] int32.
Sharding: pure data parallel, image i -> core i.

Per-core algorithm (image I [512,1024], layout A: 4 row-chunks of [128,1024]):
  - boundary detection via exp-encoded morphology on PE + ACT (exp/ln-free
    product compare), masks m3..m0 via a mask-carried dilation chain
  - 4 iterations of masked 3x3 box average with replication padding
  - separable dilated 7x7 Gaussian (dilation 6) via PE banded matmuls

Engine balance (v2): DVE was the bottleneck, so the count reciprocal runs
as an ACT spline recip, the old DVE select-mask add is folded into the
count matmul (pn = box9(m) - 16*m, Mk = relu(-2*pn+1) as int16 on ACT),
the mask H-presums run on the Pool/GPSIMD engine, and the mask/count
pipeline for iteration it+1 is emitted one iteration ahead so PE/ACT/Pool
work it while DVE finishes iteration it's value ops. The horizontal
gaussian for each chunk is emitted as soon as that chunk's final select
lands, overlapping the U-loop tail.

Host I/O is compressed for the axon tunnel: x ships as fp16, prediction as
int8, y returns as fp16 (converted back to f32 host-side). The value path
runs in fp16 on-device (DVE 2-byte fast modes); the mask/count path stays
bf16 (exact small ints). The compiled executable, weight pack, and output
scratch buffer are cached device-resident so warm calls only move x/pred
in and y out.
"""
import math
import sys
from contextlib import ExitStack

import numpy as np

sys.path.insert(0, '/opt/trn_rl_repo')

import concourse.bass as bass  # noqa: E402
import concourse.bacc as bacc  # noqa: E402
import concourse.tile as tile  # noqa: E402
from concourse import mybir  # noqa: E402

P = 128
W = 1024
H = 512
CH = 4          # row chunks
B = 8           # batch == cores
ALPHA = 4.6     # morphology exp-encoding scale
PTHR = float(np.exp(4.2))   # product threshold for boundary test
DT = mybir.dt
AF = mybir.ActivationFunctionType
OP = mybir.AluOpType


# ---------------------------------------------------------------- weights ---
def _gauss1d():
    size, sigma = 7, 1.0
    u = np.exp(-((np.arange(size) - 3.0) ** 2) / (2 * sigma ** 2))
    # 2D reference kernel is outer(u,u)/sum => separable 1D = u/sum(u)
    return (u / u.sum()).astype(np.float64)


def build_host_consts():
    """All constant weight matrices, as one dict of fp32 arrays [128,x]."""
    c = {}
    tri = np.zeros((P, P), np.float32)
    for k in range(P):
        for d in (-1, 0, 1):
            if 0 <= k + d < P:
                tri[k, k + d] = 1.0   # lhsT[k,m]: out m from in k, |k-m|<=1
    c['T_mid'] = tri
    t_top = tri.copy(); t_top[0, 0] = 2.0
    c['T_top'] = t_top
    t_bot = tri.copy(); t_bot[P - 1, P - 1] = 2.0
    c['T_bot'] = t_bot
    t_up = np.zeros((P, P), np.float32); t_up[P - 1, 0] = 1.0
    c['T_up'] = t_up
    t_dn = np.zeros((P, P), np.float32); t_dn[0, P - 1] = 1.0
    c['T_dn'] = t_dn
    c['I'] = np.eye(P, dtype=np.float32)
    c['M16'] = (-16.0 * np.eye(P)).astype(np.float32)
    bvec = np.zeros((P, P), np.float32)
    bvec[:, 0] = -4.0; bvec[0, 0] = -3.0      # bv_top
    bvec[:, 1] = -4.0; bvec[P - 1, 1] = -3.0  # bv_bot
    c['BVEC'] = bvec

    g = _gauss1d()
    for j in range(7):
        c[f'G{j}'] = (np.eye(P) * g[j]).astype(np.float16).astype(np.float32)
    # vertical gaussian: Wv[R,S] = sum_j g[j] [clamp(R+6(j-3),0,H-1)==S]
    Wv = np.zeros((H, H), np.float64)
    for R in range(H):
        for j in range(7):
            S = min(max(R + 6 * (j - 3), 0), H - 1)
            Wv[R, S] += g[j]
    for c_dst in range(CH):
        for c_src in range(CH):
            if abs(c_dst - c_src) > 1:
                continue
            blk = Wv[c_dst * P:(c_dst + 1) * P, c_src * P:(c_src + 1) * P]
            if not blk.any():
                continue
            # lhsT[k,m] = Wv[dst=128c+m, src=128c'+k]
            c[f'B_{c_dst}_{c_src}'] = (
                np.ascontiguousarray(blk.T).astype(np.float16).astype(np.float32))
    return c


# phase-M-critical weights packed first so a split wstage DMA lands them
# early; must match between _emit_once and _build_program
CRIT_W = ('I', 'T_mid', 'T_up', 'T_dn', 'T_top', 'T_bot', 'BVEC')


def _worder(consts):
    rest = sorted(n for n in consts.keys() if n not in CRIT_W)
    return list(CRIT_W) + rest


# ----------------------------------------------------------------- kernel ---
def build_kernel(ctx: ExitStack, tc: "tile.TileContext", outs, ins, reps=1):
    for _ in range(reps):
        _emit_once(ctx, tc, outs, ins)


def _emit_once(ctx: ExitStack, tc: "tile.TileContext", outs, ins):
    nc = tc.nc
    y = outs[0]                       # [512,1024] fp16 DRAM
    x, pred, wpack = ins              # x fp16, pred int8, wpack fp16 DRAM

    consts = build_host_consts()
    wnames = _worder(consts)

    if not hasattr(tc, '_bs_pools'):
        tc._bs_pools = (
            ctx.enter_context(tc.tile_pool(name="sb", bufs=1)),
            ctx.enter_context(tc.tile_pool(name="sbR", bufs=3)),
            ctx.enter_context(tc.tile_pool(name="sbM", bufs=2)),
            ctx.enter_context(tc.tile_pool(name="wp", bufs=1)),
            ctx.enter_context(tc.tile_pool(name="psB", bufs=2, space="PSUM")),
            ctx.enter_context(tc.tile_pool(name="psY", bufs=2, space="PSUM")))
    sb, sbR, sbM, wpool, psB, psY = tc._bs_pools

    # ---- persistent image buffers (chunk-blocked big tiles: one DMA each) ----
    OAbig = sb.tile([P, CH * W], DT.float16, name="OAbig", tag="OAbig")
    OBbig = sb.tile([P, CH * W], DT.float16, name="OBbig", tag="OBbig")
    OA = [OAbig[:, c * W:(c + 1) * W] for c in range(CH)]
    OB = [OBbig[:, c * W:(c + 1) * W] for c in range(CH)]
    # DMA order = consumption order: prediction feeds the phase-M exps
    # immediately (split so chunk 0's exp starts after the first half
    # lands); weights next; x is only needed at the first U iteration.
    pvbig = OBbig[:].bitcast(DT.int8)[:, 0:CH * W]
    HW2 = CH * W // 2
    nc.sync.dma_start(pvbig[:, 0:HW2], pred[:, 0:HW2])
    nc.sync.dma_start(pvbig[:, HW2:], pred[:, HW2:])
    # ACT reads the int8 labels directly in the Exp encode (no f32 staging);
    # the lab{c} tags still back the gaussian gs buffers later
    lab = [pvbig[:, c * W:(c + 1) * W] for c in range(CH)]

    # ---- load + prepare weights ----
    # split DMA: the critical block (packed first, see _worder) lands in
    # ~0.6us so PE isn't gated on the full 2.2us weight transfer
    wstage = sb.tile([P, len(wnames) * P], DT.float16, tag="wstage")
    NCRIT = len(CRIT_W) * P
    nc.sync.dma_start(wstage[:, 0:NCRIT], wpack[:, 0:NCRIT])
    nc.sync.dma_start(wstage[:, NCRIT:], wpack[:, NCRIT:len(wnames) * P])
    nc.sync.dma_start(OAbig[:], x[:, :])
    wt = {}
    BF16_W = {'T_mid', 'T_top', 'T_bot', 'T_up', 'T_dn', 'I', 'M16'}
    # phase-M-critical weights first, on DVE (idle at startup; ~94ns each)
    # so PE isn't gated on Pool's serial Q7 copy stream; everything needed
    # later (M16, R_*, G*, B_*) goes to Pool in first-use order
    for name in CRIT_W:
        if name == 'BVEC':
            continue
        i = wnames.index(name)
        t = wpool.tile([P, P], DT.bfloat16, name=f"w_{name}", tag=f"w_{name}")
        nc.vector.tensor_copy(t[:], wstage[:, i * P:(i + 1) * P])
        wt[name] = t
    for name in wnames:
        if name in CRIT_W:
            continue
        i = wnames.index(name)
        dt_w = DT.bfloat16 if name in BF16_W else DT.float16
        t = wpool.tile([P, P], dt_w, name=f"w_{name}", tag=f"w_{name}")
        nc.gpsimd.tensor_copy(t[:], wstage[:, i * P:(i + 1) * P])
        wt[name] = t
    # fp16 variants of vertical matrices for the value path
    for name in ('T_mid', 'T_top', 'T_bot', 'T_up', 'T_dn'):
        t = wpool.tile([P, P], DT.float16, name=f"wr_{name}", tag=f"wr_{name}")
        i = wnames.index(name)
        nc.gpsimd.tensor_copy(t[:], wstage[:, i * P:(i + 1) * P])
        wt['R' + name[1:]] = t

    def TRv(c):
        return wt['T_top'] if c == 0 else (wt['T_bot'] if c == CH - 1 else wt['T_mid'])

    def Rv(c):
        return wt['R_top'] if c == 0 else (wt['R_bot'] if c == CH - 1 else wt['R_mid'])

    # ---- const bias vectors ----
    def make_const(val, tag):
        t = sb.tile([P, 1], DT.float32, tag=tag)
        nc.vector.memset(t[:], val)
        return t

    b_enc_max = make_const(-9.0 * ALPHA, "b_enc_max")
    b_enc_min = make_const(+9.0 * ALPHA, "b_enc_min")
    bv_mid = make_const(-4.0, "bv_mid")
    ib = wnames.index('BVEC')
    bv_top = sb.tile([P, 1], DT.float32, name="bv_top", tag="bv_top")
    nc.vector.tensor_copy(bv_top[:], wstage[:, ib * P:ib * P + 1])
    bv_bot = sb.tile([P, 1], DT.float32, name="bv_bot", tag="bv_bot")
    nc.vector.tensor_copy(bv_bot[:], wstage[:, ib * P + 1:ib * P + 2])
    one_c = make_const(1.0, "one_c")

    def bv(c):
        return bv_top if c == 0 else (bv_bot if c == CH - 1 else bv_mid)

    GW = W + 2

    def c3(ap, cw=GW):
        # [P, CH*cw] 2D AP -> [P, CH, cw] chunk-major 3D view
        return ap.rearrange("p (c w) -> p c w", c=CH)

    def gtile(tag, dtype, guard_val, pool=sb):
        # one [P, CH*GW] tile per family: chunk views + strided guard memsets
        big = pool.tile([P, CH * GW], dtype, name=tag, tag=tag)
        nc.gpsimd.memset(big[:, 0:CH * GW:GW], guard_val)
        nc.gpsimd.memset(big[:, GW - 1:CH * GW:GW], guard_val)
        return big, [big[:, c * GW:(c + 1) * GW] for c in range(CH)]

    EmaxB, Emax = gtile("Emax", DT.bfloat16, 0.0)
    EminB, Emin = gtile("Emin", DT.bfloat16, 0.0)
    mPairs = [gtile(f"m{i}_", DT.bfloat16, 1.0) for i in range(4)]
    mB = [p[0] for p in mPairs]
    m = [p[1] for p in mPairs]
    xmB, xm = gtile("xm", DT.float16, 0.0)
    HNB = sb.tile([P, CH * W], DT.bfloat16, name="HNB", tag="HMaB")
    HN = [HNB[:, c * W:(c + 1) * W] for c in range(CH)]
    HMaB = sb.tile([P, CH * W], DT.bfloat16, name="HMaB", tag="HMaB")
    HMa = [HMaB[:, c * W:(c + 1) * W] for c in range(CH)]
    hlrB = sb.tile([P, CH * W], DT.float16, name="hlrB", tag="hlrB")
    hlr = [hlrB[:, c * W:(c + 1) * W] for c in range(CH)]

    def data(t):
        return t[:, 1:W + 1]

    def shl(t):
        return t[:, 0:W]

    def shr(t):
        return t[:, 2:W + 2]

    def pool_copy_predicated(out, mask, dat):
        eng = nc.gpsimd
        eng.add_instruction(mybir.InstCopyPredicated(
            name=f"I-{eng.bass.next_id()}",
            ins=[eng.lower_ap(mask), eng.lower_ap(dat)],
            outs=[eng.lower_ap(out)]))

    def act_recip(out, in_, bias):
        # ACT spline reciprocal: plenty accurate for 1/n of exact small
        # counts (the bass wrapper refuses Reciprocal outright, so emit
        # the instruction directly)
        eng = nc.scalar
        imm = lambda v: mybir.ImmediateValue(dtype=DT.float32, value=v)
        eng.add_instruction(mybir.InstActivation(
            name=eng.bass.get_next_instruction_name(),
            func=AF.Reciprocal,
            ins=[eng.lower_ap(in_), imm(bias), imm(1.0), imm(0.0)],
            outs=[eng.lower_ap(out)]))

    def mm_group(pt, pairs):
        # split into N=512 sub-matmuls (PSUM bank limit); weight-major order
        # so consecutive matmuls share the stationary operand (fewer LDW).
        n = pt.shape[1]
        halves = list(range(0, n, 512))
        for i, (lhsT, rhs) in enumerate(pairs):
            for h0 in halves:
                nc.tensor.matmul(pt[:, h0:h0 + 512], lhsT,
                                 rhs[:, h0:h0 + 512], start=(i == 0),
                                 stop=(i == len(pairs) - 1))

    # ================= Phase M: encode + boundary masks ===================
    for c in range(CH):
        nc.scalar.activation(data(Emax[c]), lab[c], AF.Exp,
                             bias=b_enc_max[:], scale=ALPHA)
        nc.scalar.activation(data(Emin[c]), lab[c], AF.Exp,
                             bias=b_enc_min[:], scale=-ALPHA)
    # horizontal presums (DVE, bf16 fast mode)
    SX = [sb.tile([P, W], DT.bfloat16, name=f"SX{c}", tag=f"SX{c}") for c in range(CH)]
    for c in range(CH):
        nc.vector.tensor_tensor(HN[c][:], shl(Emin[c]), shr(Emin[c]), op=OP.add)
        nc.vector.tensor_tensor(HN[c][:], HN[c][:], data(Emin[c]), op=OP.add)
        nc.vector.tensor_tensor(SX[c][:], shl(Emax[c]), shr(Emax[c]), op=OP.add)
    for c in range(CH):
        p1 = psB.tile([P, W], DT.float32, name="pS1", tag="psb")
        pairs = [(wt['T_mid'][:], data(Emax[c])),
                 (wt['I'][:], SX[c][:])]
        if c > 0:
            pairs.append((wt['T_up'][:], data(Emax[c - 1])))
        if c < CH - 1:
            pairs.append((wt['T_dn'][:], data(Emax[c + 1])))
        mm_group(p1[:], pairs)
        sc1 = sbR.tile([P, W], DT.bfloat16, name="sc1", tag="nb")
        nc.scalar.copy(sc1[:], p1[:])

        p2 = psB.tile([P, W], DT.float32, name="pS2", tag="psb")
        pairs = [(wt['T_mid'][:], HN[c][:])]
        if c > 0:
            pairs.append((wt['T_up'][:], HN[c - 1][:]))
        if c < CH - 1:
            pairs.append((wt['T_dn'][:], HN[c + 1][:]))
        mm_group(p2[:], pairs)
        pb = sbR.tile([P, W], DT.bfloat16, name="pb", tag="zt")
        nc.vector.tensor_tensor(pb[:], sc1[:], p2[:], op=OP.mult)
        nc.vector.tensor_scalar(data(m[3][c]), pb[:], PTHR, None, op0=OP.is_lt)

    # ================= Chain: m3 -> m2 -> m1 -> m0 ========================
    # (erosion semantics need guard cols = 1.0 while a mask is a chain input;
    # after its last chain use, guards are replicated for the U loop's
    # replication-padded box sums)
    for k in range(3):
        mp, mn = m[3 - k], m[2 - k]
        for c in range(CH):
            sm = sbR.tile([P, W], DT.bfloat16, name="sm", tag="sm")
            nc.vector.tensor_tensor(sm[:], shl(mp[c]), shr(mp[c]), op=OP.add)
            ps = psB.tile([P, W], DT.float32, name="pCh", tag="psb")
            pairs = [(wt['T_mid'][:], data(mp[c])),
                     (wt['I'][:], sm[:])]
            if c > 0:
                pairs.append((wt['T_up'][:], data(mp[c - 1])))
            if c < CH - 1:
                pairs.append((wt['T_dn'][:], data(mp[c + 1])))
            mm_group(ps[:], pairs)
            nc.scalar.activation(data(mn[c]), ps[:], AF.Relu, bias=bv(c)[:],
                                 scale=1.0)
        # mp fully consumed: replicate guards for the U loop (one strided
        # copy per side covers all four chunks)
        mpB = mB[3 - k]
        nc.vector.tensor_copy(mpB[:, 0:CH * GW:GW], mpB[:, 1:CH * GW:GW])
        nc.vector.tensor_copy(mpB[:, GW - 1:CH * GW:GW], mpB[:, W:CH * GW:GW])
    nc.vector.tensor_copy(mB[0][:, 0:CH * GW:GW], mB[0][:, 1:CH * GW:GW])
    nc.vector.tensor_copy(mB[0][:, GW - 1:CH * GW:GW], mB[0][:, W:CH * GW:GW])

    # ================= U loop =============================================
    GA = 18
    gs = [sb.tile([P, W + 2 * GA], DT.float16, name=f"gs{c}", tag=f"lab{c}")
          for c in range(CH)]
    hg = [sb.tile([P, W], DT.float16, name=f"Emin{c}", tag=f"Emin{c}") for c in range(CH)]
    yo = OB  # OBbig is free after the last U iteration; one output DMA

    def emit_gauss_h(c, src):
        # horizontal dilated gaussian for chunk c, emitted as soon as the
        # final U-iteration output for c lands (overlaps the U-loop tail)
        nc.vector.tensor_copy(gs[c][:, GA:GA + W], src)
        nc.vector.tensor_copy(gs[c][:, 0:GA], src[:, 0:1].to_broadcast((P, GA)))
        nc.vector.tensor_copy(gs[c][:, GA + W:],
                              src[:, W - 1:W].to_broadcast((P, GA)))
        # psb pool: the U loop's count tiles are retired by the time the
        # tail gaussian runs, so this doesn't collide with the value
        # matmuls' psy rotation
        ph = psB.tile([P, W], DT.float32, name="pH", tag="psb")
        for h in range(2):
            for j in range(7):
                off = GA + 6 * (j - 3) + h * 512
                nc.tensor.matmul(ph[:, h * 512:(h + 1) * 512], wt[f'G{j}'][:],
                                 gs[c][:, off:off + 512],
                                 start=(j == 0), stop=(j == 6))
        nc.scalar.copy(hg[c][:], ph[:])

    def emit_gauss_v(c):
        pv = psY.tile([P, W], DT.float32, name="pV", tag="psy")
        srcs = [cc for cc in range(CH) if f'B_{c}_{cc}' in wt]
        mm_group(pv[:], [(wt[f'B_{c}_{cc}'][:], hg[cc][:]) for cc in srcs])
        nc.scalar.copy(yo[c], pv[:])
        nc.sync.dma_start(y[:, c * W:(c + 1) * W], OBbig[:, c * W:(c + 1) * W])

    # chunk-merged 3D views for the U loop's elementwise stages
    xm3 = c3(xmB[:])
    xm_c = xm3[:, :, 1:W + 1]
    xm_l, xm_r = xm3[:, :, 0:W], xm3[:, :, 2:W + 2]
    hlr3 = c3(hlrB[:], W)
    HMa3 = c3(HMaB[:], W)
    m3v = [c3(t[:]) for t in mB]

    def emit_mask_path(it):
        # counts/select-mask pipeline for iteration `it`: depends ONLY on
        # the mask m[it], so it is emitted one iteration AHEAD of the value
        # path -- PE/ACT/Pool chew on it while DVE finishes the previous
        # iteration's value ops.
        mi = m[it]
        for c in range(CH):
            # Pool does shl+shr (stock Q7 op); DVE adds the center in place
            # (halved: smaller per-op DVE drains)
            nc.gpsimd.tensor_tensor(HMa[c][:], shl(mi[c]), shr(mi[c]), op=OP.add)
            for h0 in (0, 512):
                sl = slice(h0, h0 + 512)
                sg = slice(h0 + 1, h0 + 513)
                nc.vector.tensor_tensor(HMa[c][:, sl], HMa[c][:, sl],
                                        mi[c][:, sg], op=OP.add)
        MkL, nbL = [], []
        for c in range(CH):
            pn = psB.tile([P, W], DT.float32, name="pN", tag="psb")
            pairs = [(TRv(c)[:], HMa[c][:]),
                     (wt['M16'][:], data(mi[c]))]  # pn = box9(m) - 16*m
            if c > 0:
                pairs.append((wt['T_up'][:], HMa[c - 1][:]))
            if c < CH - 1:
                pairs.append((wt['T_dn'][:], HMa[c + 1][:]))
            mm_group(pn[:], pairs)
            # Mk = relu(-2*pn + 1): nonzero exactly where m==1 (pn<=-7) or
            # n==0 (pn==0); zero where m==0, n>=1 (pn>=1). Exact small ints,
            # so the int16 output (copy_predicated wants an integer mask)
            # is lossless. One ACT op replaces the old zt + DVE mask-add.
            Mk = sbM.tile([P, W], DT.int16, name="Mk", tag=f"Mk{c}")
            nc.scalar.activation(Mk[:], pn[:], AF.Relu, bias=1.0, scale=-2.0)
            MkL.append(Mk)
            # nb = 1/(pn + eps) on ACT: correct 1/n where m==0 and n>=1;
            # garbage-but-finite elsewhere (those pixels are overwritten by
            # the predicated copy below). eps keeps n==0 in the valid range.
            nb = sbM.tile([P, W], DT.float16, name="nb", tag=f"nb{c}")
            act_recip(nb[:], pn[:], 2.0 ** -40)
            nbL.append(nb)
        return MkL, nbL

    def emit_val_prep(c, src, mi):
        # xm = src*m and hlr = H3(xm) for one chunk, in 512-col halves:
        # the HW DVE pays a pipeline DRAIN ~ (dur-266ns) per op, so two
        # small drains beat one big one, and each PE half-matmul can start
        # a half earlier
        for h0 in (0, 512):
            sl = slice(h0, h0 + 512)            # W-indexed (src, hlr)
            sg = slice(h0 + 1, h0 + 513)        # xm/m data cols
            nc.vector.tensor_tensor(xm[c][:, sg], src[c][:, sl],
                                    mi[c][:, sg], op=OP.mult)
        for h0 in (0, 512):
            sl = slice(h0, h0 + 512)
            nc.vector.tensor_tensor(hlr[c][:, sl], xm[c][:, h0:h0 + 512],
                                    xm[c][:, h0 + 2:h0 + 514], op=OP.add)
        nc.vector.tensor_tensor(hlr[c][:, 0:1], hlr[c][:, 0:1],
                                xm[c][:, 1:2], op=OP.add)
        nc.vector.tensor_tensor(hlr[c][:, W - 1:W], hlr[c][:, W - 1:W],
                                xm[c][:, W:W + 1], op=OP.add)
        # the center term is NOT added here: the value matmul group carries
        # it as a second pair per stationary (V3(H3) = V3(hlr) + V3(xm)),
        # shedding one DVE op pair per chunk-iter onto PE's U-loop idle

    cur, nxt = OA, OB
    mk_nb = emit_mask_path(0)
    for c in range(CH):
        emit_val_prep(c, cur, m[0])
    for it in range(4):
        mi = m[it]
        MkL, nbL = mk_nb
        ysbL = []
        for c in range(CH):
            pyt = psY.tile([P, W], DT.float32, name="pY", tag="psy")
            pairs = [(Rv(c)[:], hlr[c][:]),
                     (Rv(c)[:], xm[c][:, 1:W + 1])]
            if c > 0:
                pairs.append((wt['R_up'][:], hlr[c - 1][:]))
                pairs.append((wt['R_up'][:], xm[c - 1][:, 1:W + 1]))
            if c < CH - 1:
                pairs.append((wt['R_dn'][:], hlr[c + 1][:]))
                pairs.append((wt['R_dn'][:], xm[c + 1][:, 1:W + 1]))
            mm_group(pyt[:], pairs)
            # avg = Y * (1/n); n==0 -> garbage, overwritten below. ACT
            # drains Y to SBUF, DVE multiplies (2-byte SBUF fast mode).
            ysb = sbR.tile([P, W], DT.float16, name="ysb", tag=f"ysb{c % 2}")
            nc.scalar.copy(ysb[:], pyt[:])
            ysbL.append(ysb)
        if it < 3:
            # next iteration's mask path, emitted here so it lands in the
            # engine queues behind this iteration's matmuls/drains
            mk_nb = emit_mask_path(it + 1)
        for c in range(CH):
            for h0 in (0, 512):
                sl = slice(h0, h0 + 512)
                nc.vector.tensor_tensor(nxt[c][:, sl], ysbL[c][:, sl],
                                        nbL[c][:, sl], op=OP.mult)
                nc.vector.copy_predicated(nxt[c][:, sl], MkL[c][:, sl],
                                          cur[c][:, sl])
            if it < 3:
                # software pipeline: this chunk's next-iteration xm/hlr
                # right after its select, so PE's next value group unblocks
                # after ~2 chunks instead of all four
                emit_val_prep(c, nxt, m[it + 1])
            else:
                # overlap the horizontal gaussian with the U-loop tail:
                # chunk c's result is final as soon as its select lands;
                # each vertical group follows as soon as its band of hg
                # rows exists, and the output DMA streams out per chunk
                emit_gauss_h(c, nxt[c])
                if c >= 1:
                    emit_gauss_v(c - 1)
        cur, nxt = nxt, cur
    emit_gauss_v(CH - 1)


# ------------------------------------------------------------ host driver ---
_CACHE = {}


def _build_program(reps=1):
    key = ('nc', reps)
    if key in _CACHE:
        return _CACHE[key], _CACHE['wpack']
    consts = build_host_consts()
    wnames = _worder(consts)
    # fp16 pack: every weight is 16-bit on device anyway; values are exact
    # small ints or already fp16-rounded, so no precision loss
    wpack = np.zeros((P, len(wnames) * P), np.float16)
    for i, n in enumerate(wnames):
        wpack[:, i * P:(i + 1) * P] = consts[n].astype(np.float16)

    nc = bacc.Bacc("TRN2", target_bir_lowering=False, debug=False,
                   num_devices=B)
    x_d = nc.dram_tensor("x", [P, CH * W], DT.float16,
                         kind="ExternalInput").ap()
    p_d = nc.dram_tensor("prediction", [P, CH * W], DT.int8,
                         kind="ExternalInput").ap()
    w_d = nc.dram_tensor("wpack", list(wpack.shape), DT.float16,
                         kind="ExternalInput").ap()
    y_d = nc.dram_tensor("y", [P, CH * W], DT.float16,
                         kind="ExternalOutput").ap()
    with tile.TileContext(nc) as tc:
        with ExitStack() as ctx:
            build_kernel(ctx, tc, [y_d], [x_d, p_d, w_d], reps=reps)
    nc.compile()
    _CACHE[('nc', reps)] = nc
    _CACHE['wpack'] = wpack
    return nc, wpack


def _get_exec(reps=1):
    """Compile (once) the 8-core sharded executable; stage constants."""
    key = ('exec', reps)
    if key in _CACHE:
        return _CACHE[key]
    import jax
    from jax.sharding import Mesh, PartitionSpec, NamedSharding
    from jax.experimental.shard_map import shard_map
    from concourse import bass2jax

    bass2jax.install_neuronx_cc_hook()
    nc, wpack = _build_program(reps)

    partition_name = (nc.partition_id_tensor.name
                      if nc.partition_id_tensor else None)
    in_names, out_names, out_avals = [], [], []
    for alloc in nc.m.functions[0].allocations:
        if not isinstance(alloc, mybir.MemoryLocationSet):
            continue
        name = alloc.memorylocations[0].name
        if alloc.kind == "ExternalInput":
            if name != partition_name:
                in_names.append(name)
        elif alloc.kind == "ExternalOutput":
            out_names.append(name)
            out_avals.append(jax.core.ShapedArray(
                tuple(alloc.tensor_shape), mybir.dt.np(alloc.dtype)))
    n_params = len(in_names)
    n_outs = len(out_names)

    devices = jax.devices()[:B]
    mesh = Mesh(np.asarray(devices), ("core",))
    shard = NamedSharding(mesh, PartitionSpec("core"))
    assert in_names == ['x', 'prediction', 'wpack'], in_names
    base_shapes = [
        jax.ShapeDtypeStruct((B * P, CH * W), np.float16, sharding=shard),
        jax.ShapeDtypeStruct((B * P, CH * W), np.int8, sharding=shard),
        jax.ShapeDtypeStruct((B * wpack.shape[0], wpack.shape[1]), np.float16,
                             sharding=shard),
    ]
    y_shape = jax.ShapeDtypeStruct((B * P, CH * W), np.float16, sharding=shard)

    # Content-address the jitted function name: the axon-side executable
    # cache can serve a stale NEFF for an unchanged module name ("jit__body")
    # even when the embedded BIR changed, so bake the program hash into the
    # module name to force an honest compile per kernel version.
    import hashlib
    bir_tag = hashlib.sha1(nc.to_json_bytes()).hexdigest()[:10]

    def make_compile_fn(with_y):
        # the kernel writes every y element, so the zero-filled y input
        # operand (run_bass_via_pjrt's donation scheme) is droppable if the
        # lowering accepts an output with no matching input operand
        all_names = list(in_names) + (list(out_names) if with_y else [])
        if partition_name is not None:
            all_names.append(partition_name)

        def _body(*args):
            operands = list(args)
            if partition_name is not None:
                operands.append(bass2jax.partition_id_tensor())
            outs = bass2jax._bass_exec_p.bind(
                *operands, out_avals=tuple(out_avals),
                in_names=tuple(all_names), out_names=tuple(out_names),
                lowering_input_output_aliases=(),
                sim_require_finite=True, sim_require_nnan=True, nc=nc)
            return tuple(outs)

        _body.__name__ = f"_body_{bir_tag}"
        _body.__qualname__ = _body.__name__
        nin = n_params + (n_outs if with_y else 0)
        arg_shapes = base_shapes + ([y_shape] * n_outs if with_y else [])

        def compile_fn():
            jf = jax.jit(shard_map(
                _body, mesh=mesh,
                in_specs=(PartitionSpec("core"),) * nin,
                out_specs=(PartitionSpec("core"),) * n_outs,
                check_rep=False), keep_unused=True)
            return jf.lower(*arg_shapes).compile()
        return compile_fn

    with_y = False
    try:
        compiled = bass2jax.fast_dispatch_compile(make_compile_fn(False))
    except Exception:
        with_y = True
        try:
            compiled = bass2jax.fast_dispatch_compile(make_compile_fn(True))
        except Exception:
            compiled = make_compile_fn(True)()

    wd = jax.device_put(np.concatenate([wpack] * B, axis=0), shard)
    extra = (wd,)
    zd = None
    if with_y:
        zd = jax.device_put(np.zeros((B * H, W), np.float16), shard)
        extra = (wd, zd)
    jax.block_until_ready(extra)

    st = {'compiled': compiled, 'shard': shard, 'wd': wd, 'zd': zd,
          'extra': extra, 'with_y': with_y, 'nc': nc, 'wpack': wpack}
    _CACHE[('exec', reps)] = st
    return st


def _stage_inputs(x, prediction):
    """Host-compress + device_put with the executable's sharding."""
    import jax
    st = _get_exec()
    # chunk-blocked per-core layout [P, CH*W]: row p holds chunks side by side
    xs = np.ascontiguousarray(
        x.reshape(B, CH, P, W).transpose(0, 2, 1, 3).reshape(B * P, CH * W)
    ).astype(np.float16)
    ps = np.ascontiguousarray(
        prediction.reshape(B, CH, P, W).transpose(0, 2, 1, 3)
        .reshape(B * P, CH * W)).astype(np.int8)
    xd = jax.device_put(xs, st['shard'])
    pd = jax.device_put(ps, st['shard'])
    return xd, pd


def _unpack_y(arr):
    """[B*P, CH*W] fp16 chunk-blocked -> [B,1,H,W] f32."""
    return (np.asarray(arr).astype(np.float32)
            .reshape(B, P, CH, W).transpose(0, 2, 1, 3).reshape(B, 1, H, W))


def kernel(x: np.ndarray, prediction: np.ndarray) -> np.ndarray:
    st = _get_exec()
    xd, pd = _stage_inputs(x, prediction)
    out = st['compiled'](xd, pd, *st['extra'])
    return _unpack_y(out[0])


if __name__ == "__main__":
    xs = np.random.randn(B, 1, H, W).astype(np.float32)
    ps = np.random.randint(0, 19, size=(B, 1, H, W)).astype(np.int32)
    print(kernel(xs, ps).shape)



# revision 55
# speedup vs baseline: 1.1705x; 1.0981x over previous
"""Trainium2 Bass kernel for nn_BoundarySuppressionWithSmoothing.

Full inputs: x [8,1,512,1024] f32, prediction [8,1,512,1024] int32.
Sharding: pure data parallel, image i -> core i.

Per-core algorithm (image I [512,1024], layout A: 4 row-chunks of [128,1024]):
  - boundary detection via exp-encoded morphology on PE + ACT (exp/ln-free
    product compare), masks m3..m0 via a mask-carried dilation chain
  - 4 iterations of masked 3x3 box average with replication padding
  - separable dilated 7x7 Gaussian (dilation 6) via PE banded matmuls

Engine balance (v2): DVE was the bottleneck, so the count reciprocal runs
as an ACT spline recip, the old DVE select-mask add is folded into the
count matmul (pn = box9(m) - 16*m, Mk = relu(-2*pn+1) as int16 on ACT),
the mask H-presums run on the Pool/GPSIMD engine, and the mask/count
pipeline for iteration it+1 is emitted one iteration ahead so PE/ACT/Pool
work it while DVE finishes iteration it's value ops. The horizontal
gaussian for each chunk is emitted as soon as that chunk's final select
lands, overlapping the U-loop tail.

Host I/O is compressed for the axon tunnel: x ships as fp16, prediction as
int8, y returns as fp16 (converted back to f32 host-side). The value path
runs in fp16 on-device (DVE 2-byte fast modes); the mask/count path stays
bf16 (exact small ints). The compiled executable, weight pack, and output
scratch buffer are cached device-resident so warm calls only move x/pred
in and y out.
"""
import math
import sys
from contextlib import ExitStack

import numpy as np

sys.path.insert(0, '/opt/trn_rl_repo')

import concourse.bass as bass  # noqa: E402
import concourse.bacc as bacc  # noqa: E402
import concourse.tile as tile  # noqa: E402
from concourse import mybir  # noqa: E402

P = 128
W = 1024
H = 512
CH = 4          # row chunks
B = 8           # batch == cores
ALPHA = 4.6     # morphology exp-encoding scale
PTHR = float(np.exp(4.2))   # product threshold for boundary test
DT = mybir.dt
AF = mybir.ActivationFunctionType
OP = mybir.AluOpType


# ---------------------------------------------------------------- weights ---
def _gauss1d():
    size, sigma = 7, 1.0
    u = np.exp(-((np.arange(size) - 3.0) ** 2) / (2 * sigma ** 2))
    # 2D reference kernel is outer(u,u)/sum => separable 1D = u/sum(u)
    return (u / u.sum()).astype(np.float64)


def build_host_consts():
    """All constant weight matrices, as one dict of fp32 arrays [128,x]."""
    c = {}
    tri = np.zeros((P, P), np.float32)
    for k in range(P):
        for d in (-1, 0, 1):
            if 0 <= k + d < P:
                tri[k, k + d] = 1.0   # lhsT[k,m]: out m from in k, |k-m|<=1
    c['T_mid'] = tri
    t_top = tri.copy(); t_top[0, 0] = 2.0
    c['T_top'] = t_top
    t_bot = tri.copy(); t_bot[P - 1, P - 1] = 2.0
    c['T_bot'] = t_bot
    t_up = np.zeros((P, P), np.float32); t_up[P - 1, 0] = 1.0
    c['T_up'] = t_up
    t_dn = np.zeros((P, P), np.float32); t_dn[0, P - 1] = 1.0
    c['T_dn'] = t_dn
    c['I'] = np.eye(P, dtype=np.float32)
    c['M16'] = (-16.0 * np.eye(P)).astype(np.float32)
    bvec = np.zeros((P, P), np.float32)
    bvec[:, 0] = -4.0; bvec[0, 0] = -3.0      # bv_top
    bvec[:, 1] = -4.0; bvec[P - 1, 1] = -3.0  # bv_bot
    c['BVEC'] = bvec

    g = _gauss1d()
    for j in range(7):
        c[f'G{j}'] = (np.eye(P) * g[j]).astype(np.float16).astype(np.float32)
    # vertical gaussian: Wv[R,S] = sum_j g[j] [clamp(R+6(j-3),0,H-1)==S]
    Wv = np.zeros((H, H), np.float64)
    for R in range(H):
        for j in range(7):
            S = min(max(R + 6 * (j - 3), 0), H - 1)
            Wv[R, S] += g[j]
    for c_dst in range(CH):
        for c_src in range(CH):
            if abs(c_dst - c_src) > 1:
                continue
            blk = Wv[c_dst * P:(c_dst + 1) * P, c_src * P:(c_src + 1) * P]
            if not blk.any():
                continue
            # lhsT[k,m] = Wv[dst=128c+m, src=128c'+k]
            c[f'B_{c_dst}_{c_src}'] = (
                np.ascontiguousarray(blk.T).astype(np.float16).astype(np.float32))
    return c


# phase-M-critical weights packed first so a split wstage DMA lands them
# early; must match between _emit_once and _build_program
CRIT_W = ('I', 'T_mid', 'T_up', 'T_dn', 'T_top', 'T_bot', 'BVEC')


def _worder(consts):
    rest = sorted(n for n in consts.keys() if n not in CRIT_W)
    return list(CRIT_W) + rest


# ----------------------------------------------------------------- kernel ---
def build_kernel(ctx: ExitStack, tc: "tile.TileContext", outs, ins, reps=1):
    for _ in range(reps):
        _emit_once(ctx, tc, outs, ins)


def _emit_once(ctx: ExitStack, tc: "tile.TileContext", outs, ins):
    nc = tc.nc
    y = outs[0]                       # [512,1024] fp16 DRAM
    x, pred, wpack = ins              # x fp16, pred int8, wpack fp16 DRAM

    consts = build_host_consts()
    wnames = _worder(consts)

    if not hasattr(tc, '_bs_pools'):
        tc._bs_pools = (
            ctx.enter_context(tc.tile_pool(name="sb", bufs=1)),
            ctx.enter_context(tc.tile_pool(name="sbR", bufs=3)),
            ctx.enter_context(tc.tile_pool(name="sbM", bufs=2)),
            ctx.enter_context(tc.tile_pool(name="wp", bufs=1)),
            ctx.enter_context(tc.tile_pool(name="psB", bufs=2, space="PSUM")),
            ctx.enter_context(tc.tile_pool(name="psY", bufs=2, space="PSUM")))
    sb, sbR, sbM, wpool, psB, psY = tc._bs_pools

    # ---- persistent image buffers (chunk-blocked big tiles: one DMA each) ----
    OAbig = sb.tile([P, CH * W], DT.float16, name="OAbig", tag="OAbig")
    OBbig = sb.tile([P, CH * W], DT.float16, name="OBbig", tag="OBbig")
    OA = [OAbig[:, c * W:(c + 1) * W] for c in range(CH)]
    OB = [OBbig[:, c * W:(c + 1) * W] for c in range(CH)]
    # DMA order = consumption order: prediction feeds the phase-M exps
    # immediately (split so chunk 0's exp starts after the first half
    # lands); weights next; x is only needed at the first U iteration.
    pvbig = OBbig[:].bitcast(DT.int8)[:, 0:CH * W]
    HW2 = CH * W // 2
    nc.sync.dma_start(pvbig[:, 0:HW2], pred[:, 0:HW2])
    nc.sync.dma_start(pvbig[:, HW2:], pred[:, HW2:])
    # ACT reads the int8 labels directly in the Exp encode (no f32 staging);
    # the lab{c} tags still back the gaussian gs buffers later
    lab = [pvbig[:, c * W:(c + 1) * W] for c in range(CH)]

    # ---- load + prepare weights ----
    # split DMA: the critical block (packed first, see _worder) lands in
    # ~0.6us so PE isn't gated on the full 2.2us weight transfer
    wstage = sb.tile([P, len(wnames) * P], DT.float16, tag="wstage")
    NCRIT = len(CRIT_W) * P
    nc.sync.dma_start(wstage[:, 0:NCRIT], wpack[:, 0:NCRIT])
    nc.sync.dma_start(wstage[:, NCRIT:], wpack[:, NCRIT:len(wnames) * P])
    nc.sync.dma_start(OAbig[:], x[:, :])
    wt = {}
    BF16_W = {'T_mid', 'T_top', 'T_bot', 'T_up', 'T_dn', 'I', 'M16'}
    # phase-M-critical weights first, on DVE (idle at startup; ~94ns each)
    # so PE isn't gated on Pool's serial Q7 copy stream; everything needed
    # later (M16, R_*, G*, B_*) goes to Pool in first-use order
    for name in CRIT_W:
        if name == 'BVEC':
            continue
        i = wnames.index(name)
        t = wpool.tile([P, P], DT.bfloat16, name=f"w_{name}", tag=f"w_{name}")
        nc.vector.tensor_copy(t[:], wstage[:, i * P:(i + 1) * P])
        wt[name] = t
    for name in wnames:
        if name in CRIT_W:
            continue
        i = wnames.index(name)
        dt_w = DT.bfloat16 if name in BF16_W else DT.float16
        t = wpool.tile([P, P], dt_w, name=f"w_{name}", tag=f"w_{name}")
        nc.gpsimd.tensor_copy(t[:], wstage[:, i * P:(i + 1) * P])
        wt[name] = t
    # fp16 variants of vertical matrices for the value path
    for name in ('T_mid', 'T_top', 'T_bot', 'T_up', 'T_dn'):
        t = wpool.tile([P, P], DT.float16, name=f"wr_{name}", tag=f"wr_{name}")
        i = wnames.index(name)
        nc.gpsimd.tensor_copy(t[:], wstage[:, i * P:(i + 1) * P])
        wt['R' + name[1:]] = t

    def TRv(c):
        return wt['T_top'] if c == 0 else (wt['T_bot'] if c == CH - 1 else wt['T_mid'])

    def Rv(c):
        return wt['R_top'] if c == 0 else (wt['R_bot'] if c == CH - 1 else wt['R_mid'])

    # ---- const bias vectors ----
    def make_const(val, tag):
        t = sb.tile([P, 1], DT.float32, tag=tag)
        nc.vector.memset(t[:], val)
        return t

    b_enc_max = make_const(-9.0 * ALPHA, "b_enc_max")
    b_enc_min = make_const(+9.0 * ALPHA, "b_enc_min")
    bv_mid = make_const(-4.0, "bv_mid")
    ib = wnames.index('BVEC')
    bv_top = sb.tile([P, 1], DT.float32, name="bv_top", tag="bv_top")
    nc.vector.tensor_copy(bv_top[:], wstage[:, ib * P:ib * P + 1])
    bv_bot = sb.tile([P, 1], DT.float32, name="bv_bot", tag="bv_bot")
    nc.vector.tensor_copy(bv_bot[:], wstage[:, ib * P + 1:ib * P + 2])
    one_c = make_const(1.0, "one_c")

    def bv(c):
        return bv_top if c == 0 else (bv_bot if c == CH - 1 else bv_mid)

    GW = W + 2

    def c3(ap, cw=GW):
        # [P, CH*cw] 2D AP -> [P, CH, cw] chunk-major 3D view
        return ap.rearrange("p (c w) -> p c w", c=CH)

    def gtile(tag, dtype, guard_val, pool=sb):
        # one [P, CH*GW] tile per family: chunk views + strided guard memsets
        big = pool.tile([P, CH * GW], dtype, name=tag, tag=tag)
        nc.gpsimd.memset(big[:, 0:CH * GW:GW], guard_val)
        nc.gpsimd.memset(big[:, GW - 1:CH * GW:GW], guard_val)
        return big, [big[:, c * GW:(c + 1) * GW] for c in range(CH)]

    EmaxB, Emax = gtile("Emax", DT.bfloat16, 0.0)
    EminB, Emin = gtile("Emin", DT.bfloat16, 0.0)
    mPairs = [gtile(f"m{i}_", DT.bfloat16, 1.0) for i in range(4)]
    mB = [p[0] for p in mPairs]
    m = [p[1] for p in mPairs]
    xmB, xm = gtile("xm", DT.float16, 0.0)
    HNB = sb.tile([P, CH * W], DT.bfloat16, name="HNB", tag="HMaB")
    HN = [HNB[:, c * W:(c + 1) * W] for c in range(CH)]
    HMaB = sb.tile([P, CH * W], DT.bfloat16, name="HMaB", tag="HMaB")
    HMa = [HMaB[:, c * W:(c + 1) * W] for c in range(CH)]
    hlrB = sb.tile([P, CH * W], DT.float16, name="hlrB", tag="hlrB")
    hlr = [hlrB[:, c * W:(c + 1) * W] for c in range(CH)]

    def data(t):
        return t[:, 1:W + 1]

    def shl(t):
        return t[:, 0:W]

    def shr(t):
        return t[:, 2:W + 2]

    def pool_copy_predicated(out, mask, dat):
        eng = nc.gpsimd
        eng.add_instruction(mybir.InstCopyPredicated(
            name=f"I-{eng.bass.next_id()}",
            ins=[eng.lower_ap(mask), eng.lower_ap(dat)],
            outs=[eng.lower_ap(out)]))

    def act_recip(out, in_, bias):
        # ACT spline reciprocal: plenty accurate for 1/n of exact small
        # counts (the bass wrapper refuses Reciprocal outright, so emit
        # the instruction directly)
        eng = nc.scalar
        imm = lambda v: mybir.ImmediateValue(dtype=DT.float32, value=v)
        eng.add_instruction(mybir.InstActivation(
            name=eng.bass.get_next_instruction_name(),
            func=AF.Reciprocal,
            ins=[eng.lower_ap(in_), imm(bias), imm(1.0), imm(0.0)],
            outs=[eng.lower_ap(out)]))

    def mm_group(pt, pairs):
        # split into N=512 sub-matmuls (PSUM bank limit); weight-major order
        # so consecutive matmuls share the stationary operand (fewer LDW).
        n = pt.shape[1]
        halves = list(range(0, n, 512))
        for i, (lhsT, rhs) in enumerate(pairs):
            for h0 in halves:
                nc.tensor.matmul(pt[:, h0:h0 + 512], lhsT,
                                 rhs[:, h0:h0 + 512], start=(i == 0),
                                 stop=(i == len(pairs) - 1))

    # ================= Phase M: encode + boundary masks ===================
    for c in range(CH):
        nc.scalar.activation(data(Emax[c]), lab[c], AF.Exp,
                             bias=b_enc_max[:], scale=ALPHA)
        nc.scalar.activation(data(Emin[c]), lab[c], AF.Exp,
                             bias=b_enc_min[:], scale=-ALPHA)
    # horizontal presums (DVE, bf16 fast mode)
    SX = [sb.tile([P, W], DT.bfloat16, name=f"SX{c}", tag=f"SX{c}") for c in range(CH)]
    for c in range(CH):
        nc.vector.tensor_tensor(HN[c][:], shl(Emin[c]), shr(Emin[c]), op=OP.add)
        nc.vector.tensor_tensor(HN[c][:], HN[c][:], data(Emin[c]), op=OP.add)
        nc.vector.tensor_tensor(SX[c][:], shl(Emax[c]), shr(Emax[c]), op=OP.add)
    for c in range(CH):
        p1 = psB.tile([P, W], DT.float32, name="pS1", tag="psb")
        pairs = [(wt['T_mid'][:], data(Emax[c])),
                 (wt['I'][:], SX[c][:])]
        if c > 0:
            pairs.append((wt['T_up'][:], data(Emax[c - 1])))
        if c < CH - 1:
            pairs.append((wt['T_dn'][:], data(Emax[c + 1])))
        mm_group(p1[:], pairs)
        sc1 = sbR.tile([P, W], DT.bfloat16, name="sc1", tag="nb")
        nc.scalar.copy(sc1[:], p1[:])

        p2 = psB.tile([P, W], DT.float32, name="pS2", tag="psb")
        pairs = [(wt['T_mid'][:], HN[c][:])]
        if c > 0:
            pairs.append((wt['T_up'][:], HN[c - 1][:]))
        if c < CH - 1:
            pairs.append((wt['T_dn'][:], HN[c + 1][:]))
        mm_group(p2[:], pairs)
        pb = sbR.tile([P, W], DT.bfloat16, name="pb", tag="zt")
        nc.vector.tensor_tensor(pb[:], sc1[:], p2[:], op=OP.mult)
        nc.vector.tensor_scalar(data(m[3][c]), pb[:], PTHR, None, op0=OP.is_lt)

    # ================= Chain: m3 -> m2 -> m1 -> m0 ========================
    # (erosion semantics need guard cols = 1.0 while a mask is a chain input;
    # after its last chain use, guards are replicated for the U loop's
    # replication-padded box sums)
    for k in range(3):
        mp, mn = m[3 - k], m[2 - k]
        for c in range(CH):
            sm = sbR.tile([P, W], DT.bfloat16, name="sm", tag="sm")
            nc.vector.tensor_tensor(sm[:], shl(mp[c]), shr(mp[c]), op=OP.add)
            ps = psB.tile([P, W], DT.float32, name="pCh", tag="psb")
            pairs = [(wt['T_mid'][:], data(mp[c])),
                     (wt['I'][:], sm[:])]
            if c > 0:
                pairs.append((wt['T_up'][:], data(mp[c - 1])))
            if c < CH - 1:
                pairs.append((wt['T_dn'][:], data(mp[c + 1])))
            mm_group(ps[:], pairs)
            nc.scalar.activation(data(mn[c]), ps[:], AF.Relu, bias=bv(c)[:],
                                 scale=1.0)
        # mp fully consumed: replicate guards for the U loop (one strided
        # copy per side covers all four chunks)
        mpB = mB[3 - k]
        nc.vector.tensor_copy(mpB[:, 0:CH * GW:GW], mpB[:, 1:CH * GW:GW])
        nc.vector.tensor_copy(mpB[:, GW - 1:CH * GW:GW], mpB[:, W:CH * GW:GW])
    nc.vector.tensor_copy(mB[0][:, 0:CH * GW:GW], mB[0][:, 1:CH * GW:GW])
    nc.vector.tensor_copy(mB[0][:, GW - 1:CH * GW:GW], mB[0][:, W:CH * GW:GW])

    # ================= U loop =============================================
    GA = 18
    gs = [sb.tile([P, W + 2 * GA], DT.float16, name=f"gs{c}", tag=f"lab{c}")
          for c in range(CH)]
    hg = [sb.tile([P, W], DT.float16, name=f"Emin{c}", tag=f"Emin{c}") for c in range(CH)]
    yo = OB  # OBbig is free after the last U iteration; one output DMA

    def emit_gauss_h(c, src):
        # horizontal dilated gaussian for chunk c, emitted as soon as the
        # final U-iteration output for c lands (overlaps the U-loop tail)
        nc.vector.tensor_copy(gs[c][:, GA:GA + W], src)
        nc.vector.tensor_copy(gs[c][:, 0:GA], src[:, 0:1].to_broadcast((P, GA)))
        nc.vector.tensor_copy(gs[c][:, GA + W:],
                              src[:, W - 1:W].to_broadcast((P, GA)))
        # psb pool: the U loop's count tiles are retired by the time the
        # tail gaussian runs, so this doesn't collide with the value
        # matmuls' psy rotation
        ph = psB.tile([P, W], DT.float32, name="pH", tag="psb")
        for h in range(2):
            for j in range(7):
                off = GA + 6 * (j - 3) + h * 512
                nc.tensor.matmul(ph[:, h * 512:(h + 1) * 512], wt[f'G{j}'][:],
                                 gs[c][:, off:off + 512],
                                 start=(j == 0), stop=(j == 6))
        nc.scalar.copy(hg[c][:], ph[:])

    def emit_gauss_v(c):
        pv = psY.tile([P, W], DT.float32, name="pV", tag="psy")
        srcs = [cc for cc in range(CH) if f'B_{c}_{cc}' in wt]
        mm_group(pv[:], [(wt[f'B_{c}_{cc}'][:], hg[cc][:]) for cc in srcs])
        nc.scalar.copy(yo[c], pv[:])
        nc.sync.dma_start(y[:, c * W:(c + 1) * W], OBbig[:, c * W:(c + 1) * W])

    # chunk-merged 3D views for the U loop's elementwise stages
    xm3 = c3(xmB[:])
    xm_c = xm3[:, :, 1:W + 1]
    xm_l, xm_r = xm3[:, :, 0:W], xm3[:, :, 2:W + 2]
    hlr3 = c3(hlrB[:], W)
    HMa3 = c3(HMaB[:], W)
    m3v = [c3(t[:]) for t in mB]

    def emit_mask_path(it):
        # counts/select-mask pipeline for iteration `it`: depends ONLY on
        # the mask m[it], so it is emitted one iteration AHEAD of the value
        # path -- PE/ACT/Pool chew on it while DVE finishes the previous
        # iteration's value ops.
        mi = m[it]
        for c in range(CH):
            # Pool does shl+shr (stock Q7 op); DVE adds the center in place
            # (halved: smaller per-op DVE drains)
            nc.gpsimd.tensor_tensor(HMa[c][:], shl(mi[c]), shr(mi[c]), op=OP.add)
            for h0 in (0, 512):
                sl = slice(h0, h0 + 512)
                sg = slice(h0 + 1, h0 + 513)
                nc.vector.tensor_tensor(HMa[c][:, sl], HMa[c][:, sl],
                                        mi[c][:, sg], op=OP.add)
        MkL, nbL = [], []
        for c in range(CH):
            pn = psB.tile([P, W], DT.float32, name="pN", tag="psb")
            pairs = [(TRv(c)[:], HMa[c][:]),
                     (wt['M16'][:], data(mi[c]))]  # pn = box9(m) - 16*m
            if c > 0:
                pairs.append((wt['T_up'][:], HMa[c - 1][:]))
            if c < CH - 1:
                pairs.append((wt['T_dn'][:], HMa[c + 1][:]))
            mm_group(pn[:], pairs)
            # Mk = relu(-2*pn + 1): nonzero exactly where m==1 (pn<=-7) or
            # n==0 (pn==0); zero where m==0, n>=1 (pn>=1). Exact small ints,
            # so the int16 output (copy_predicated wants an integer mask)
            # is lossless. One ACT op replaces the old zt + DVE mask-add.
            Mk = sbM.tile([P, W], DT.int16, name="Mk", tag=f"Mk{c}")
            nc.scalar.activation(Mk[:], pn[:], AF.Relu, bias=1.0, scale=-2.0)
            MkL.append(Mk)
            # nb = 1/(pn + eps) on ACT: correct 1/n where m==0 and n>=1;
            # garbage-but-finite elsewhere (those pixels are overwritten by
            # the predicated copy below). eps keeps n==0 in the valid range.
            nb = sbM.tile([P, W], DT.float16, name="nb", tag=f"nb{c}")
            act_recip(nb[:], pn[:], 2.0 ** -40)
            nbL.append(nb)
        return MkL, nbL

    def emit_val_prep(c, src, mi):
        # xm = src*m and hlr = H3(xm) for one chunk, in 512-col halves:
        # the HW DVE pays a pipeline DRAIN ~ (dur-266ns) per op, so two
        # small drains beat one big one, and each PE half-matmul can start
        # a half earlier
        for h0 in (0, 512):
            sl = slice(h0, h0 + 512)            # W-indexed (src, hlr)
            sg = slice(h0 + 1, h0 + 513)        # xm/m data cols
            nc.vector.tensor_tensor(xm[c][:, sg], src[c][:, sl],
                                    mi[c][:, sg], op=OP.mult)
        for h0 in (0, 512):
            sl = slice(h0, h0 + 512)
            nc.vector.tensor_tensor(hlr[c][:, sl], xm[c][:, h0:h0 + 512],
                                    xm[c][:, h0 + 2:h0 + 514], op=OP.add)
        nc.vector.tensor_tensor(hlr[c][:, 0:1], hlr[c][:, 0:1],
                                xm[c][:, 1:2], op=OP.add)
        nc.vector.tensor_tensor(hlr[c][:, W - 1:W], hlr[c][:, W - 1:W],
                                xm[c][:, W:W + 1], op=OP.add)
        for h0 in (0, 512):
            sl = slice(h0, h0 + 512)
            sg = slice(h0 + 1, h0 + 513)
            nc.vector.tensor_tensor(hlr[c][:, sl], hlr[c][:, sl],
                                    xm[c][:, sg], op=OP.add)

    cur, nxt = OA, OB
    mk_nb = emit_mask_path(0)
    for c in range(CH):
        emit_val_prep(c, cur, m[0])
    for it in range(4):
        mi = m[it]
        MkL, nbL = mk_nb
        ysbL = []
        for c in range(CH):
            pyt = psY.tile([P, W], DT.float32, name="pY", tag="psy")
            pairs = [(Rv(c)[:], hlr[c][:])]
            if c > 0:
                pairs.append((wt['R_up'][:], hlr[c - 1][:]))
            if c < CH - 1:
                pairs.append((wt['R_dn'][:], hlr[c + 1][:]))
            mm_group(pyt[:], pairs)
            # avg = Y * (1/n); n==0 -> garbage, overwritten below. ACT
            # drains Y to SBUF, DVE multiplies (2-byte SBUF fast mode).
            ysb = sbR.tile([P, W], DT.float16, name="ysb", tag=f"ysb{c % 2}")
            nc.scalar.copy(ysb[:], pyt[:])
            ysbL.append(ysb)
        if it < 3:
            # next iteration's mask path, emitted here so it lands in the
            # engine queues behind this iteration's matmuls/drains
            mk_nb = emit_mask_path(it + 1)
        for c in range(CH):
            for h0 in (0, 512):
                sl = slice(h0, h0 + 512)
                nc.vector.tensor_tensor(nxt[c][:, sl], ysbL[c][:, sl],
                                        nbL[c][:, sl], op=OP.mult)
                nc.vector.copy_predicated(nxt[c][:, sl], MkL[c][:, sl],
                                          cur[c][:, sl])
            if it < 3:
                # software pipeline: this chunk's next-iteration xm/hlr
                # right after its select, so PE's next value group unblocks
                # after ~2 chunks instead of all four
                emit_val_prep(c, nxt, m[it + 1])
            else:
                # overlap the horizontal gaussian with the U-loop tail:
                # chunk c's result is final as soon as its select lands;
                # each vertical group follows as soon as its band of hg
                # rows exists, and the output DMA streams out per chunk
                emit_gauss_h(c, nxt[c])
                if c >= 1:
                    emit_gauss_v(c - 1)
        cur, nxt = nxt, cur
    emit_gauss_v(CH - 1)


# ------------------------------------------------------------ host driver ---
_CACHE = {}


def _build_program(reps=1):
    key = ('nc', reps)
    if key in _CACHE:
        return _CACHE[key], _CACHE['wpack']
    consts = build_host_consts()
    wnames = _worder(consts)
    # fp16 pack: every weight is 16-bit on device anyway; values are exact
    # small ints or already fp16-rounded, so no precision loss
    wpack = np.zeros((P, len(wnames) * P), np.float16)
    for i, n in enumerate(wnames):
        wpack[:, i * P:(i + 1) * P] = consts[n].astype(np.float16)

    nc = bacc.Bacc("TRN2", target_bir_lowering=False, debug=False,
                   num_devices=B)
    x_d = nc.dram_tensor("x", [P, CH * W], DT.float16,
                         kind="ExternalInput").ap()
    p_d = nc.dram_tensor("prediction", [P, CH * W], DT.int8,
                         kind="ExternalInput").ap()
    w_d = nc.dram_tensor("wpack", list(wpack.shape), DT.float16,
                         kind="ExternalInput").ap()
    y_d = nc.dram_tensor("y", [P, CH * W], DT.float16,
                         kind="ExternalOutput").ap()
    with tile.TileContext(nc) as tc:
        with ExitStack() as ctx:
            build_kernel(ctx, tc, [y_d], [x_d, p_d, w_d], reps=reps)
    nc.compile()
    _CACHE[('nc', reps)] = nc
    _CACHE['wpack'] = wpack
    return nc, wpack


def _get_exec(reps=1):
    """Compile (once) the 8-core sharded executable; stage constants."""
    key = ('exec', reps)
    if key in _CACHE:
        return _CACHE[key]
    import jax
    from jax.sharding import Mesh, PartitionSpec, NamedSharding
    from jax.experimental.shard_map import shard_map
    from concourse import bass2jax

    bass2jax.install_neuronx_cc_hook()
    nc, wpack = _build_program(reps)

    partition_name = (nc.partition_id_tensor.name
                      if nc.partition_id_tensor else None)
    in_names, out_names, out_avals = [], [], []
    for alloc in nc.m.functions[0].allocations:
        if not isinstance(alloc, mybir.MemoryLocationSet):
            continue
        name = alloc.memorylocations[0].name
        if alloc.kind == "ExternalInput":
            if name != partition_name:
                in_names.append(name)
        elif alloc.kind == "ExternalOutput":
            out_names.append(name)
            out_avals.append(jax.core.ShapedArray(
                tuple(alloc.tensor_shape), mybir.dt.np(alloc.dtype)))
    n_params = len(in_names)
    n_outs = len(out_names)

    devices = jax.devices()[:B]
    mesh = Mesh(np.asarray(devices), ("core",))
    shard = NamedSharding(mesh, PartitionSpec("core"))
    assert in_names == ['x', 'prediction', 'wpack'], in_names
    base_shapes = [
        jax.ShapeDtypeStruct((B * P, CH * W), np.float16, sharding=shard),
        jax.ShapeDtypeStruct((B * P, CH * W), np.int8, sharding=shard),
        jax.ShapeDtypeStruct((B * wpack.shape[0], wpack.shape[1]), np.float16,
                             sharding=shard),
    ]
    y_shape = jax.ShapeDtypeStruct((B * P, CH * W), np.float16, sharding=shard)

    # Content-address the jitted function name: the axon-side executable
    # cache can serve a stale NEFF for an unchanged module name ("jit__body")
    # even when the embedded BIR changed, so bake the program hash into the
    # module name to force an honest compile per kernel version.
    import hashlib
    bir_tag = hashlib.sha1(nc.to_json_bytes()).hexdigest()[:10]

    def make_compile_fn(with_y):
        # the kernel writes every y element, so the zero-filled y input
        # operand (run_bass_via_pjrt's donation scheme) is droppable if the
        # lowering accepts an output with no matching input operand
        all_names = list(in_names) + (list(out_names) if with_y else [])
        if partition_name is not None:
            all_names.append(partition_name)

        def _body(*args):
            operands = list(args)
            if partition_name is not None:
                operands.append(bass2jax.partition_id_tensor())
            outs = bass2jax._bass_exec_p.bind(
                *operands, out_avals=tuple(out_avals),
                in_names=tuple(all_names), out_names=tuple(out_names),
                lowering_input_output_aliases=(),
                sim_require_finite=True, sim_require_nnan=True, nc=nc)
            return tuple(outs)

        _body.__name__ = f"_body_{bir_tag}"
        _body.__qualname__ = _body.__name__
        nin = n_params + (n_outs if with_y else 0)
        arg_shapes = base_shapes + ([y_shape] * n_outs if with_y else [])

        def compile_fn():
            jf = jax.jit(shard_map(
                _body, mesh=mesh,
                in_specs=(PartitionSpec("core"),) * nin,
                out_specs=(PartitionSpec("core"),) * n_outs,
                check_rep=False), keep_unused=True)
            return jf.lower(*arg_shapes).compile()
        return compile_fn

    with_y = False
    try:
        compiled = bass2jax.fast_dispatch_compile(make_compile_fn(False))
    except Exception:
        with_y = True
        try:
            compiled = bass2jax.fast_dispatch_compile(make_compile_fn(True))
        except Exception:
            compiled = make_compile_fn(True)()

    wd = jax.device_put(np.concatenate([wpack] * B, axis=0), shard)
    extra = (wd,)
    zd = None
    if with_y:
        zd = jax.device_put(np.zeros((B * H, W), np.float16), shard)
        extra = (wd, zd)
    jax.block_until_ready(extra)

    st = {'compiled': compiled, 'shard': shard, 'wd': wd, 'zd': zd,
          'extra': extra, 'with_y': with_y, 'nc': nc, 'wpack': wpack}
    _CACHE[('exec', reps)] = st
    return st


def _stage_inputs(x, prediction):
    """Host-compress + device_put with the executable's sharding."""
    import jax
    st = _get_exec()
    # chunk-blocked per-core layout [P, CH*W]: row p holds chunks side by side
    xs = np.ascontiguousarray(
        x.reshape(B, CH, P, W).transpose(0, 2, 1, 3).reshape(B * P, CH * W)
    ).astype(np.float16)
    ps = np.ascontiguousarray(
        prediction.reshape(B, CH, P, W).transpose(0, 2, 1, 3)
        .reshape(B * P, CH * W)).astype(np.int8)
    xd = jax.device_put(xs, st['shard'])
    pd = jax.device_put(ps, st['shard'])
    return xd, pd


def _unpack_y(arr):
    """[B*P, CH*W] fp16 chunk-blocked -> [B,1,H,W] f32."""
    return (np.asarray(arr).astype(np.float32)
            .reshape(B, P, CH, W).transpose(0, 2, 1, 3).reshape(B, 1, H, W))


def kernel(x: np.ndarray, prediction: np.ndarray) -> np.ndarray:
    st = _get_exec()
    xd, pd = _stage_inputs(x, prediction)
    out = st['compiled'](xd, pd, *st['extra'])
    return _unpack_y(out[0])


if __name__ == "__main__":
    xs = np.random.randn(B, 1, H, W).astype(np.float32)
    ps = np.random.randint(0, 19, size=(B, 1, H, W)).astype(np.int32)
    print(kernel(xs, ps).shape)



# revision 58
# speedup vs baseline: 1.3292x; 1.1356x over previous
"""Trainium2 Bass kernel for nn_BoundarySuppressionWithSmoothing.

Full inputs: x [8,1,512,1024] f32, prediction [8,1,512,1024] int32.
Sharding: pure data parallel, image i -> core i.

Per-core algorithm (image I [512,1024], layout A: 4 row-chunks of [128,1024]):
  - boundary detection via exp-encoded morphology on PE + ACT (exp/ln-free
    product compare), masks m3..m0 via a mask-carried dilation chain
  - 4 iterations of masked 3x3 box average with replication padding
  - separable dilated 7x7 Gaussian (dilation 6) via PE banded matmuls

Engine balance (v2): DVE was the bottleneck, so the count reciprocal runs
as an ACT spline recip, the old DVE select-mask add is folded into the
count matmul (pn = box9(m) - 16*m, Mk = relu(-2*pn+1) as int16 on ACT),
the mask H-presums run on the Pool/GPSIMD engine, and the mask/count
pipeline for iteration it+1 is emitted one iteration ahead so PE/ACT/Pool
work it while DVE finishes iteration it's value ops. The horizontal
gaussian for each chunk is emitted as soon as that chunk's final select
lands, overlapping the U-loop tail.

Host I/O is compressed for the axon tunnel: x ships as fp16, prediction as
int8, y returns as fp16 (converted back to f32 host-side). The value path
runs in fp16 on-device (DVE 2-byte fast modes); the mask/count path stays
bf16 (exact small ints). The compiled executable, weight pack, and output
scratch buffer are cached device-resident so warm calls only move x/pred
in and y out.
"""
import math
import sys
from contextlib import ExitStack

import numpy as np

sys.path.insert(0, '/opt/trn_rl_repo')

import concourse.bass as bass  # noqa: E402
import concourse.bacc as bacc  # noqa: E402
import concourse.tile as tile  # noqa: E402
from concourse import mybir  # noqa: E402

P = 128
W = 1024
H = 512
CH = 4          # row chunks
B = 8           # batch == cores
ALPHA = 4.6     # morphology exp-encoding scale
PTHR = float(np.exp(4.2))   # product threshold for boundary test
DT = mybir.dt
AF = mybir.ActivationFunctionType
OP = mybir.AluOpType


# ---------------------------------------------------------------- weights ---
def _gauss1d():
    size, sigma = 7, 1.0
    u = np.exp(-((np.arange(size) - 3.0) ** 2) / (2 * sigma ** 2))
    # 2D reference kernel is outer(u,u)/sum => separable 1D = u/sum(u)
    return (u / u.sum()).astype(np.float64)


def build_host_consts():
    """All constant weight matrices, as one dict of fp32 arrays [128,x]."""
    c = {}
    tri = np.zeros((P, P), np.float32)
    for k in range(P):
        for d in (-1, 0, 1):
            if 0 <= k + d < P:
                tri[k, k + d] = 1.0   # lhsT[k,m]: out m from in k, |k-m|<=1
    c['T_mid'] = tri
    t_top = tri.copy(); t_top[0, 0] = 2.0
    c['T_top'] = t_top
    t_bot = tri.copy(); t_bot[P - 1, P - 1] = 2.0
    c['T_bot'] = t_bot
    t_up = np.zeros((P, P), np.float32); t_up[P - 1, 0] = 1.0
    c['T_up'] = t_up
    t_dn = np.zeros((P, P), np.float32); t_dn[0, P - 1] = 1.0
    c['T_dn'] = t_dn
    c['I'] = np.eye(P, dtype=np.float32)
    c['M16'] = (-16.0 * np.eye(P)).astype(np.float32)
    bvec = np.zeros((P, P), np.float32)
    bvec[:, 0] = -4.0; bvec[0, 0] = -3.0      # bv_top
    bvec[:, 1] = -4.0; bvec[P - 1, 1] = -3.0  # bv_bot
    c['BVEC'] = bvec

    g = _gauss1d()
    for j in range(7):
        c[f'G{j}'] = (np.eye(P) * g[j]).astype(np.float16).astype(np.float32)
    # vertical gaussian: Wv[R,S] = sum_j g[j] [clamp(R+6(j-3),0,H-1)==S]
    Wv = np.zeros((H, H), np.float64)
    for R in range(H):
        for j in range(7):
            S = min(max(R + 6 * (j - 3), 0), H - 1)
            Wv[R, S] += g[j]
    for c_dst in range(CH):
        for c_src in range(CH):
            if abs(c_dst - c_src) > 1:
                continue
            blk = Wv[c_dst * P:(c_dst + 1) * P, c_src * P:(c_src + 1) * P]
            if not blk.any():
                continue
            # lhsT[k,m] = Wv[dst=128c+m, src=128c'+k]
            c[f'B_{c_dst}_{c_src}'] = (
                np.ascontiguousarray(blk.T).astype(np.float16).astype(np.float32))
    return c


# phase-M-critical weights packed first so a split wstage DMA lands them
# early; must match between _emit_once and _build_program
CRIT_W = ('I', 'T_mid', 'T_up', 'T_dn', 'T_top', 'T_bot', 'BVEC')


def _worder(consts):
    rest = sorted(n for n in consts.keys() if n not in CRIT_W)
    return list(CRIT_W) + rest


# ----------------------------------------------------------------- kernel ---
def build_kernel(ctx: ExitStack, tc: "tile.TileContext", outs, ins, reps=1):
    for _ in range(reps):
        _emit_once(ctx, tc, outs, ins)


def _emit_once(ctx: ExitStack, tc: "tile.TileContext", outs, ins):
    nc = tc.nc
    y = outs[0]                       # [512,1024] fp16 DRAM
    x, pred, wpack = ins              # x fp16, pred int8, wpack fp16 DRAM

    consts = build_host_consts()
    wnames = _worder(consts)

    if not hasattr(tc, '_bs_pools'):
        tc._bs_pools = (
            ctx.enter_context(tc.tile_pool(name="sb", bufs=1)),
            ctx.enter_context(tc.tile_pool(name="sbR", bufs=3)),
            ctx.enter_context(tc.tile_pool(name="sbM", bufs=2)),
            ctx.enter_context(tc.tile_pool(name="wp", bufs=1)),
            ctx.enter_context(tc.tile_pool(name="psB", bufs=2, space="PSUM")),
            ctx.enter_context(tc.tile_pool(name="psY", bufs=2, space="PSUM")))
    sb, sbR, sbM, wpool, psB, psY = tc._bs_pools

    # ---- persistent image buffers (chunk-blocked big tiles: one DMA each) ----
    OAbig = sb.tile([P, CH * W], DT.float16, name="OAbig", tag="OAbig")
    OBbig = sb.tile([P, CH * W], DT.float16, name="OBbig", tag="OBbig")
    OA = [OAbig[:, c * W:(c + 1) * W] for c in range(CH)]
    OB = [OBbig[:, c * W:(c + 1) * W] for c in range(CH)]
    # DMA order = consumption order: prediction feeds the phase-M exps
    # immediately (split so chunk 0's exp starts after the first half
    # lands); weights next; x is only needed at the first U iteration.
    pvbig = OBbig[:].bitcast(DT.int8)[:, 0:CH * W]
    HW2 = CH * W // 2
    nc.sync.dma_start(pvbig[:, 0:HW2], pred[:, 0:HW2])
    nc.sync.dma_start(pvbig[:, HW2:], pred[:, HW2:])
    # ACT reads the int8 labels directly in the Exp encode (no f32 staging);
    # the lab{c} tags still back the gaussian gs buffers later
    lab = [pvbig[:, c * W:(c + 1) * W] for c in range(CH)]

    # ---- load + prepare weights ----
    # split DMA: the critical block (packed first, see _worder) lands in
    # ~0.6us so PE isn't gated on the full 2.2us weight transfer
    wstage = sb.tile([P, len(wnames) * P], DT.float16, tag="wstage")
    NCRIT = len(CRIT_W) * P
    nc.sync.dma_start(wstage[:, 0:NCRIT], wpack[:, 0:NCRIT])
    nc.sync.dma_start(wstage[:, NCRIT:], wpack[:, NCRIT:len(wnames) * P])
    nc.sync.dma_start(OAbig[:], x[:, :])
    wt = {}
    BF16_W = {'T_mid', 'T_top', 'T_bot', 'T_up', 'T_dn', 'I', 'M16'}
    # phase-M-critical weights first, on DVE (idle at startup; ~94ns each)
    # so PE isn't gated on Pool's serial Q7 copy stream; everything needed
    # later (M16, R_*, G*, B_*) goes to Pool in first-use order
    for name in CRIT_W:
        if name == 'BVEC':
            continue
        i = wnames.index(name)
        t = wpool.tile([P, P], DT.bfloat16, name=f"w_{name}", tag=f"w_{name}")
        nc.vector.tensor_copy(t[:], wstage[:, i * P:(i + 1) * P])
        wt[name] = t
    for name in wnames:
        if name in CRIT_W:
            continue
        i = wnames.index(name)
        dt_w = DT.bfloat16 if name in BF16_W else DT.float16
        t = wpool.tile([P, P], dt_w, name=f"w_{name}", tag=f"w_{name}")
        nc.gpsimd.tensor_copy(t[:], wstage[:, i * P:(i + 1) * P])
        wt[name] = t
    # fp16 variants of vertical matrices for the value path
    for name in ('T_mid', 'T_top', 'T_bot', 'T_up', 'T_dn'):
        t = wpool.tile([P, P], DT.float16, name=f"wr_{name}", tag=f"wr_{name}")
        i = wnames.index(name)
        nc.gpsimd.tensor_copy(t[:], wstage[:, i * P:(i + 1) * P])
        wt['R' + name[1:]] = t

    def TRv(c):
        return wt['T_top'] if c == 0 else (wt['T_bot'] if c == CH - 1 else wt['T_mid'])

    def Rv(c):
        return wt['R_top'] if c == 0 else (wt['R_bot'] if c == CH - 1 else wt['R_mid'])

    # ---- const bias vectors ----
    def make_const(val, tag):
        t = sb.tile([P, 1], DT.float32, tag=tag)
        nc.vector.memset(t[:], val)
        return t

    b_enc_max = make_const(-9.0 * ALPHA, "b_enc_max")
    b_enc_min = make_const(+9.0 * ALPHA, "b_enc_min")
    bv_mid = make_const(-4.0, "bv_mid")
    ib = wnames.index('BVEC')
    bv_top = sb.tile([P, 1], DT.float32, name="bv_top", tag="bv_top")
    nc.vector.tensor_copy(bv_top[:], wstage[:, ib * P:ib * P + 1])
    bv_bot = sb.tile([P, 1], DT.float32, name="bv_bot", tag="bv_bot")
    nc.vector.tensor_copy(bv_bot[:], wstage[:, ib * P + 1:ib * P + 2])
    one_c = make_const(1.0, "one_c")

    def bv(c):
        return bv_top if c == 0 else (bv_bot if c == CH - 1 else bv_mid)

    GW = W + 2

    def c3(ap, cw=GW):
        # [P, CH*cw] 2D AP -> [P, CH, cw] chunk-major 3D view
        return ap.rearrange("p (c w) -> p c w", c=CH)

    def gtile(tag, dtype, guard_val, pool=sb):
        # one [P, CH*GW] tile per family: chunk views + strided guard memsets
        big = pool.tile([P, CH * GW], dtype, name=tag, tag=tag)
        nc.gpsimd.memset(big[:, 0:CH * GW:GW], guard_val)
        nc.gpsimd.memset(big[:, GW - 1:CH * GW:GW], guard_val)
        return big, [big[:, c * GW:(c + 1) * GW] for c in range(CH)]

    EmaxB, Emax = gtile("Emax", DT.bfloat16, 0.0)
    EminB, Emin = gtile("Emin", DT.bfloat16, 0.0)
    mPairs = [gtile(f"m{i}_", DT.bfloat16, 1.0) for i in range(4)]
    mB = [p[0] for p in mPairs]
    m = [p[1] for p in mPairs]
    xmB, xm = gtile("xm", DT.float16, 0.0)
    HNB = sb.tile([P, CH * W], DT.bfloat16, name="HNB", tag="HMaB")
    HN = [HNB[:, c * W:(c + 1) * W] for c in range(CH)]
    HMaB = sb.tile([P, CH * W], DT.bfloat16, name="HMaB", tag="HMaB")
    HMa = [HMaB[:, c * W:(c + 1) * W] for c in range(CH)]
    hlrB = sb.tile([P, CH * W], DT.float16, name="hlrB", tag="hlrB")
    hlr = [hlrB[:, c * W:(c + 1) * W] for c in range(CH)]

    def data(t):
        return t[:, 1:W + 1]

    def shl(t):
        return t[:, 0:W]

    def shr(t):
        return t[:, 2:W + 2]

    def pool_copy_predicated(out, mask, dat):
        eng = nc.gpsimd
        eng.add_instruction(mybir.InstCopyPredicated(
            name=f"I-{eng.bass.next_id()}",
            ins=[eng.lower_ap(mask), eng.lower_ap(dat)],
            outs=[eng.lower_ap(out)]))

    def act_recip(out, in_, bias):
        # ACT spline reciprocal: plenty accurate for 1/n of exact small
        # counts (the bass wrapper refuses Reciprocal outright, so emit
        # the instruction directly)
        eng = nc.scalar
        imm = lambda v: mybir.ImmediateValue(dtype=DT.float32, value=v)
        eng.add_instruction(mybir.InstActivation(
            name=eng.bass.get_next_instruction_name(),
            func=AF.Reciprocal,
            ins=[eng.lower_ap(in_), imm(bias), imm(1.0), imm(0.0)],
            outs=[eng.lower_ap(out)]))

    def mm_group(pt, pairs):
        # split into N=512 sub-matmuls (PSUM bank limit); weight-major order
        # so consecutive matmuls share the stationary operand (fewer LDW).
        n = pt.shape[1]
        halves = list(range(0, n, 512))
        for i, (lhsT, rhs) in enumerate(pairs):
            for h0 in halves:
                nc.tensor.matmul(pt[:, h0:h0 + 512], lhsT,
                                 rhs[:, h0:h0 + 512], start=(i == 0),
                                 stop=(i == len(pairs) - 1))

    # ================= Phase M: encode + boundary masks ===================
    for c in range(CH):
        nc.scalar.activation(data(Emax[c]), lab[c], AF.Exp,
                             bias=b_enc_max[:], scale=ALPHA)
        nc.scalar.activation(data(Emin[c]), lab[c], AF.Exp,
                             bias=b_enc_min[:], scale=-ALPHA)
    # horizontal presums (DVE, bf16 fast mode)
    SX = [sb.tile([P, W], DT.bfloat16, name=f"SX{c}", tag=f"SX{c}") for c in range(CH)]
    for c in range(CH):
        nc.vector.tensor_tensor(HN[c][:], shl(Emin[c]), shr(Emin[c]), op=OP.add)
        nc.vector.tensor_tensor(HN[c][:], HN[c][:], data(Emin[c]), op=OP.add)
        nc.vector.tensor_tensor(SX[c][:], shl(Emax[c]), shr(Emax[c]), op=OP.add)
    for c in range(CH):
        p1 = psB.tile([P, W], DT.float32, name="pS1", tag="psb")
        pairs = [(wt['T_mid'][:], data(Emax[c])),
                 (wt['I'][:], SX[c][:])]
        if c > 0:
            pairs.append((wt['T_up'][:], data(Emax[c - 1])))
        if c < CH - 1:
            pairs.append((wt['T_dn'][:], data(Emax[c + 1])))
        mm_group(p1[:], pairs)
        sc1 = sbR.tile([P, W], DT.bfloat16, name="sc1", tag="nb")
        nc.scalar.copy(sc1[:], p1[:])

        p2 = psB.tile([P, W], DT.float32, name="pS2", tag="psb")
        pairs = [(wt['T_mid'][:], HN[c][:])]
        if c > 0:
            pairs.append((wt['T_up'][:], HN[c - 1][:]))
        if c < CH - 1:
            pairs.append((wt['T_dn'][:], HN[c + 1][:]))
        mm_group(p2[:], pairs)
        pb = sbR.tile([P, W], DT.bfloat16, name="pb", tag="zt")
        nc.vector.tensor_tensor(pb[:], sc1[:], p2[:], op=OP.mult)
        nc.vector.tensor_scalar(data(m[3][c]), pb[:], PTHR, None, op0=OP.is_lt)

    # ================= Chain: m3 -> m2 -> m1 -> m0 ========================
    # (erosion semantics need guard cols = 1.0 while a mask is a chain input;
    # after its last chain use, guards are replicated for the U loop's
    # replication-padded box sums)
    for k in range(3):
        mp, mn = m[3 - k], m[2 - k]
        for c in range(CH):
            sm = sbR.tile([P, W], DT.bfloat16, name="sm", tag="sm")
            nc.vector.tensor_tensor(sm[:], shl(mp[c]), shr(mp[c]), op=OP.add)
            ps = psB.tile([P, W], DT.float32, name="pCh", tag="psb")
            pairs = [(wt['T_mid'][:], data(mp[c])),
                     (wt['I'][:], sm[:])]
            if c > 0:
                pairs.append((wt['T_up'][:], data(mp[c - 1])))
            if c < CH - 1:
                pairs.append((wt['T_dn'][:], data(mp[c + 1])))
            mm_group(ps[:], pairs)
            nc.scalar.activation(data(mn[c]), ps[:], AF.Relu, bias=bv(c)[:],
                                 scale=1.0)
        # mp fully consumed: replicate guards for the U loop (one strided
        # copy per side covers all four chunks)
        mpB = mB[3 - k]
        nc.vector.tensor_copy(mpB[:, 0:CH * GW:GW], mpB[:, 1:CH * GW:GW])
        nc.vector.tensor_copy(mpB[:, GW - 1:CH * GW:GW], mpB[:, W:CH * GW:GW])
    nc.vector.tensor_copy(mB[0][:, 0:CH * GW:GW], mB[0][:, 1:CH * GW:GW])
    nc.vector.tensor_copy(mB[0][:, GW - 1:CH * GW:GW], mB[0][:, W:CH * GW:GW])

    # ================= U loop =============================================
    GA = 18
    gs = [sb.tile([P, W + 2 * GA], DT.float16, name=f"gs{c}", tag=f"lab{c}")
          for c in range(CH)]
    hg = [sb.tile([P, W], DT.float16, name=f"Emin{c}", tag=f"Emin{c}") for c in range(CH)]
    yo = OB  # OBbig is free after the last U iteration; one output DMA

    def emit_gauss_h(c, src):
        # horizontal dilated gaussian for chunk c, emitted as soon as the
        # final U-iteration output for c lands (overlaps the U-loop tail)
        nc.vector.tensor_copy(gs[c][:, GA:GA + W], src)
        nc.vector.tensor_copy(gs[c][:, 0:GA], src[:, 0:1].to_broadcast((P, GA)))
        nc.vector.tensor_copy(gs[c][:, GA + W:],
                              src[:, W - 1:W].to_broadcast((P, GA)))
        # psb pool: the U loop's count tiles are retired by the time the
        # tail gaussian runs, so this doesn't collide with the value
        # matmuls' psy rotation
        ph = psB.tile([P, W], DT.float32, name="pH", tag="psb")
        for h in range(2):
            for j in range(7):
                off = GA + 6 * (j - 3) + h * 512
                nc.tensor.matmul(ph[:, h * 512:(h + 1) * 512], wt[f'G{j}'][:],
                                 gs[c][:, off:off + 512],
                                 start=(j == 0), stop=(j == 6))
        nc.scalar.copy(hg[c][:], ph[:])

    def emit_gauss_v(c):
        pv = psY.tile([P, W], DT.float32, name="pV", tag="psy")
        srcs = [cc for cc in range(CH) if f'B_{c}_{cc}' in wt]
        mm_group(pv[:], [(wt[f'B_{c}_{cc}'][:], hg[cc][:]) for cc in srcs])
        nc.scalar.copy(yo[c], pv[:])
        nc.sync.dma_start(y[:, c * W:(c + 1) * W], OBbig[:, c * W:(c + 1) * W])

    # chunk-merged 3D views for the U loop's elementwise stages
    xm3 = c3(xmB[:])
    xm_c = xm3[:, :, 1:W + 1]
    xm_l, xm_r = xm3[:, :, 0:W], xm3[:, :, 2:W + 2]
    hlr3 = c3(hlrB[:], W)
    HMa3 = c3(HMaB[:], W)
    m3v = [c3(t[:]) for t in mB]

    def emit_mask_path(it):
        # counts/select-mask pipeline for iteration `it`: depends ONLY on
        # the mask m[it], so it is emitted one iteration AHEAD of the value
        # path -- PE/ACT/Pool chew on it while DVE finishes the previous
        # iteration's value ops.
        mi = m[it]
        for c in range(CH):
            # Pool does shl+shr (stock Q7 op); DVE adds the center in place
            # (halved: smaller per-op DVE drains)
            nc.gpsimd.tensor_tensor(HMa[c][:], shl(mi[c]), shr(mi[c]), op=OP.add)
            for h0 in (0, 512):
                sl = slice(h0, h0 + 512)
                sg = slice(h0 + 1, h0 + 513)
                nc.vector.tensor_tensor(HMa[c][:, sl], HMa[c][:, sl],
                                        mi[c][:, sg], op=OP.add)
        MkL, nbL = [], []
        for c in range(CH):
            pn = psB.tile([P, W], DT.float32, name="pN", tag="psb")
            pairs = [(TRv(c)[:], HMa[c][:]),
                     (wt['M16'][:], data(mi[c]))]  # pn = box9(m) - 16*m
            if c > 0:
                pairs.append((wt['T_up'][:], HMa[c - 1][:]))
            if c < CH - 1:
                pairs.append((wt['T_dn'][:], HMa[c + 1][:]))
            mm_group(pn[:], pairs)
            # Mk = relu(-2*pn + 1): nonzero exactly where m==1 (pn<=-7) or
            # n==0 (pn==0); zero where m==0, n>=1 (pn>=1). Exact small ints,
            # so the int16 output (copy_predicated wants an integer mask)
            # is lossless. One ACT op replaces the old zt + DVE mask-add.
            Mk = sbM.tile([P, W], DT.int16, name="Mk", tag=f"Mk{c}")
            nc.scalar.activation(Mk[:], pn[:], AF.Relu, bias=1.0, scale=-2.0)
            MkL.append(Mk)
            # nb = 1/(pn + eps) on ACT: correct 1/n where m==0 and n>=1;
            # garbage-but-finite elsewhere (those pixels are overwritten by
            # the predicated copy below). eps keeps n==0 in the valid range.
            nb = sbM.tile([P, W], DT.float16, name="nb", tag=f"nb{c}")
            act_recip(nb[:], pn[:], 2.0 ** -40)
            nbL.append(nb)
        return MkL, nbL

    def emit_val_prep(c, src, mi):
        # xm = src*m and hlr = H3(xm) for one chunk, in 512-col halves:
        # the HW DVE pays a pipeline DRAIN ~ (dur-266ns) per op, so two
        # small drains beat one big one, and each PE half-matmul can start
        # a half earlier
        for h0 in (0, 512):
            sl = slice(h0, h0 + 512)            # W-indexed (src, hlr)
            sg = slice(h0 + 1, h0 + 513)        # xm/m data cols
            nc.vector.tensor_tensor(xm[c][:, sg], src[c][:, sl],
                                    mi[c][:, sg], op=OP.mult)
        for h0 in (0, 512):
            sl = slice(h0, h0 + 512)
            nc.vector.tensor_tensor(hlr[c][:, sl], xm[c][:, h0:h0 + 512],
                                    xm[c][:, h0 + 2:h0 + 514], op=OP.add)
        nc.vector.tensor_tensor(hlr[c][:, 0:1], hlr[c][:, 0:1],
                                xm[c][:, 1:2], op=OP.add)
        nc.vector.tensor_tensor(hlr[c][:, W - 1:W], hlr[c][:, W - 1:W],
                                xm[c][:, W:W + 1], op=OP.add)
        for h0 in (0, 512):
            sl = slice(h0, h0 + 512)
            sg = slice(h0 + 1, h0 + 513)
            nc.vector.tensor_tensor(hlr[c][:, sl], hlr[c][:, sl],
                                    xm[c][:, sg], op=OP.add)

    cur, nxt = OA, OB
    mk_nb = emit_mask_path(0)
    for c in range(CH):
        emit_val_prep(c, cur, m[0])
    for it in range(4):
        mi = m[it]
        MkL, nbL = mk_nb
        ysbL = []
        for c in range(CH):
            pyt = psY.tile([P, W], DT.float32, name="pY", tag="psy")
            pairs = [(Rv(c)[:], hlr[c][:])]
            if c > 0:
                pairs.append((wt['R_up'][:], hlr[c - 1][:]))
            if c < CH - 1:
                pairs.append((wt['R_dn'][:], hlr[c + 1][:]))
            mm_group(pyt[:], pairs)
            # avg = Y * (1/n); n==0 -> garbage, overwritten below. ACT
            # drains Y to SBUF, DVE multiplies (2-byte SBUF fast mode).
            ysb = sbR.tile([P, W], DT.float16, name="ysb", tag=f"ysb{c % 2}")
            nc.scalar.copy(ysb[:], pyt[:])
            ysbL.append(ysb)
        if it < 3:
            # next iteration's mask path, emitted here so it lands in the
            # engine queues behind this iteration's matmuls/drains
            mk_nb = emit_mask_path(it + 1)
        for c in range(CH):
            for h0 in (0, 512):
                sl = slice(h0, h0 + 512)
                nc.vector.tensor_tensor(nxt[c][:, sl], ysbL[c][:, sl],
                                        nbL[c][:, sl], op=OP.mult)
                nc.vector.copy_predicated(nxt[c][:, sl], MkL[c][:, sl],
                                          cur[c][:, sl])
            if it < 3:
                # software pipeline: this chunk's next-iteration xm/hlr
                # right after its select, so PE's next value group unblocks
                # after ~2 chunks instead of all four
                emit_val_prep(c, nxt, m[it + 1])
            else:
                # overlap the horizontal gaussian with the U-loop tail:
                # chunk c's result is final as soon as its select lands;
                # each vertical group follows as soon as its band of hg
                # rows exists, and the output DMA streams out per chunk
                emit_gauss_h(c, nxt[c])
                if c >= 1:
                    emit_gauss_v(c - 1)
        cur, nxt = nxt, cur
    emit_gauss_v(CH - 1)


# ------------------------------------------------------------ host driver ---
_CACHE = {}


def _build_program(reps=1):
    key = ('nc', reps)
    if key in _CACHE:
        return _CACHE[key], _CACHE['wpack']
    consts = build_host_consts()
    wnames = _worder(consts)
    # fp16 pack: every weight is 16-bit on device anyway; values are exact
    # small ints or already fp16-rounded, so no precision loss
    wpack = np.zeros((P, len(wnames) * P), np.float16)
    for i, n in enumerate(wnames):
        wpack[:, i * P:(i + 1) * P] = consts[n].astype(np.float16)

    nc = bacc.Bacc("TRN2", target_bir_lowering=False, debug=False,
                   num_devices=B)
    x_d = nc.dram_tensor("x", [P, CH * W], DT.float16,
                         kind="ExternalInput").ap()
    p_d = nc.dram_tensor("prediction", [P, CH * W], DT.int8,
                         kind="ExternalInput").ap()
    # weights ship inside the NEFF (Const + ant_data): one fewer runtime
    # operand to marshal per dispatch, no device_put staging
    w_d = nc.inline_tensor(wpack, name="wpack").ap()
    y_d = nc.dram_tensor("y", [P, CH * W], DT.float16,
                         kind="ExternalOutput").ap()
    with tile.TileContext(nc) as tc:
        with ExitStack() as ctx:
            build_kernel(ctx, tc, [y_d], [x_d, p_d, w_d], reps=reps)
    nc.compile()
    _CACHE[('nc', reps)] = nc
    _CACHE['wpack'] = wpack
    return nc, wpack


def _get_exec(reps=1):
    """Compile (once) the 8-core sharded executable; stage constants."""
    key = ('exec', reps)
    if key in _CACHE:
        return _CACHE[key]
    import jax
    from jax.sharding import Mesh, PartitionSpec, NamedSharding
    from jax.experimental.shard_map import shard_map
    from concourse import bass2jax

    bass2jax.install_neuronx_cc_hook()
    nc, wpack = _build_program(reps)

    partition_name = (nc.partition_id_tensor.name
                      if nc.partition_id_tensor else None)
    in_names, out_names, out_avals = [], [], []
    for alloc in nc.m.functions[0].allocations:
        if not isinstance(alloc, mybir.MemoryLocationSet):
            continue
        name = alloc.memorylocations[0].name
        if alloc.kind == "ExternalInput":
            if name != partition_name:
                in_names.append(name)
        elif alloc.kind == "ExternalOutput":
            out_names.append(name)
            out_avals.append(jax.core.ShapedArray(
                tuple(alloc.tensor_shape), mybir.dt.np(alloc.dtype)))
    n_params = len(in_names)
    n_outs = len(out_names)

    devices = jax.devices()[:B]
    mesh = Mesh(np.asarray(devices), ("core",))
    shard = NamedSharding(mesh, PartitionSpec("core"))
    assert in_names == ['x', 'prediction'], in_names
    base_shapes = [
        jax.ShapeDtypeStruct((B * P, CH * W), np.float16, sharding=shard),
        jax.ShapeDtypeStruct((B * P, CH * W), np.int8, sharding=shard),
    ]
    y_shape = jax.ShapeDtypeStruct((B * P, CH * W), np.float16, sharding=shard)

    # Content-address the jitted function name: the axon-side executable
    # cache can serve a stale NEFF for an unchanged module name ("jit__body")
    # even when the embedded BIR changed, so bake the program hash into the
    # module name to force an honest compile per kernel version.
    import hashlib
    bir_tag = hashlib.sha1(nc.to_json_bytes()).hexdigest()[:10]

    def make_compile_fn(with_y):
        # the kernel writes every y element, so the zero-filled y input
        # operand (run_bass_via_pjrt's donation scheme) is droppable if the
        # lowering accepts an output with no matching input operand
        all_names = list(in_names) + (list(out_names) if with_y else [])
        if partition_name is not None:
            all_names.append(partition_name)

        def _body(*args):
            operands = list(args)
            if partition_name is not None:
                operands.append(bass2jax.partition_id_tensor())
            outs = bass2jax._bass_exec_p.bind(
                *operands, out_avals=tuple(out_avals),
                in_names=tuple(all_names), out_names=tuple(out_names),
                lowering_input_output_aliases=(),
                sim_require_finite=True, sim_require_nnan=True, nc=nc)
            return tuple(outs)

        _body.__name__ = f"_body_{bir_tag}"
        _body.__qualname__ = _body.__name__
        nin = n_params + (n_outs if with_y else 0)
        arg_shapes = base_shapes + ([y_shape] * n_outs if with_y else [])

        def compile_fn():
            jf = jax.jit(shard_map(
                _body, mesh=mesh,
                in_specs=(PartitionSpec("core"),) * nin,
                out_specs=(PartitionSpec("core"),) * n_outs,
                check_rep=False), keep_unused=True)
            return jf.lower(*arg_shapes).compile()
        return compile_fn

    with_y = False
    try:
        compiled = bass2jax.fast_dispatch_compile(make_compile_fn(False))
    except Exception:
        with_y = True
        try:
            compiled = bass2jax.fast_dispatch_compile(make_compile_fn(True))
        except Exception:
            compiled = make_compile_fn(True)()

    extra = ()
    zd = None
    if with_y:
        zd = jax.device_put(np.zeros((B * H, W), np.float16), shard)
        extra = (zd,)
        jax.block_until_ready(extra)

    st = {'compiled': compiled, 'shard': shard, 'zd': zd,
          'extra': extra, 'with_y': with_y, 'nc': nc, 'wpack': wpack}
    _CACHE[('exec', reps)] = st
    return st


def _stage_inputs(x, prediction):
    """Host-compress + device_put with the executable's sharding."""
    import jax
    st = _get_exec()
    # chunk-blocked per-core layout [P, CH*W]: row p holds chunks side by side
    xs = np.ascontiguousarray(
        x.reshape(B, CH, P, W).transpose(0, 2, 1, 3).reshape(B * P, CH * W)
    ).astype(np.float16)
    ps = np.ascontiguousarray(
        prediction.reshape(B, CH, P, W).transpose(0, 2, 1, 3)
        .reshape(B * P, CH * W)).astype(np.int8)
    xd = jax.device_put(xs, st['shard'])
    pd = jax.device_put(ps, st['shard'])
    return xd, pd


def _unpack_y(arr):
    """[B*P, CH*W] fp16 chunk-blocked -> [B,1,H,W] f32."""
    return (np.asarray(arr).astype(np.float32)
            .reshape(B, P, CH, W).transpose(0, 2, 1, 3).reshape(B, 1, H, W))


def kernel(x: np.ndarray, prediction: np.ndarray) -> np.ndarray:
    st = _get_exec()
    xd, pd = _stage_inputs(x, prediction)
    out = st['compiled'](xd, pd, *st['extra'])
    return _unpack_y(out[0])


if __name__ == "__main__":
    xs = np.random.randn(B, 1, H, W).astype(np.float32)
    ps = np.random.randint(0, 19, size=(B, 1, H, W)).astype(np.int32)
    print(kernel(xs, ps).shape)

